# revision 1
# baseline (speedup 1.0000x reference)
"""Trainium2 Bass kernel for nn_Encoder: 6-layer post-LN transformer encoder.

Sharding: pure data-parallel over batch across 8 NeuronCores (2 sequences per
core), zero collectives. On-device layout is feature-major ([D on partitions,
tokens on free dim]) so every projection uses the stored weight directly as the
matmul stationary operand and per-feature biases are per-partition scalars.

Attention computes transposed scores [t, s] per head (K=64 matmuls on partition
halves), exponentiates without max-subtraction (scores are O(1) by
construction; masking is exp(s)*(1-m), exact since exp(-1e9) underflows to 0),
and contracts PV with col-tiled matmuls writing the two heads of a pair into
the two partition halves of one PSUM bank. Softmax denominators come from an
M=1 ones matmul; normalization is broadcast back over partitions with a K=1
outer product on the PE.

LayerNorm reduces over the feature (partition) axis with ones-matmuls
(E[x^2]-E[x]^2+eps), then applies (z*rstd)*g + (-g*mean*rstd + b) where the
per-token row factors are broadcast across partitions via K=1/K=2 PE outer
products and per-feature factors are per-partition scalars.
"""

import os
import sys

import numpy as np

sys.path.insert(0, "/opt/trn_rl_repo")

import concourse.bass as bass  # noqa: E402
import concourse.mybir as mybir  # noqa: E402
import concourse.tile as tile  # noqa: E402
from concourse import bacc  # noqa: E402
from concourse.bass_utils import run_bass_kernel_spmd  # noqa: E402
from concourse.masks import make_identity  # noqa: E402

# Problem constants (hardcoded per harness contract).
V, D, H, F = 32000, 768, 12, 3072
L = int(os.environ.get("ENC_LAYERS", "6"))
DN = D // H            # 64
B, S = 16, 512
NCORES = 8
BL = B // NCORES       # 2 sequences per core
T = BL * S             # 1024 tokens per core
P = 128
DT = D // P            # 6 feature tiles
TC = T // P            # 8 token chunks
SC = S // P            # 4 chunks per sequence
FT = F // P            # 24 ff tiles
NCH = 2                # T split into chunks of 512 for matmul free dim
CH = T // NCH          # 512
REPS = int(os.environ.get("ENC_REPS", "1"))  # timing: rerun layers in-NEFF
NODMAW = os.environ.get("ENC_NODMA_W", "0") == "1"  # debug: skip weight DMA
SKIP = set(os.environ.get("ENC_SKIP", "").split(","))  # debug: skip phases
EPS = 1e-5
FP32 = mybir.dt.float32
FP32R = mybir.dt.float32r
I32 = mybir.dt.int32

AF = mybir.ActivationFunctionType
OP = mybir.AluOpType

_PROGRAM_CACHE = {}


def _build_program():
    nc = bacc.Bacc("TRN2", target_bir_lowering=False, debug=False,
                   num_devices=NCORES)

    io = {}

    def inp(name, shape, dtype=FP32):
        io[name] = nc.declare_dram_parameter(name, list(shape), dtype,
                                             isOutput=False)

    inp("x_idx", [TC, P], I32)
    inp("emb", [V, D])
    inp("pe", [P, DT, S])
    inp("mmask", [P, BL, SC, S])
    inp("wq", [L, DT, DT, P, P], FP32R)   # [l, ktile, mtile, 128k, 128m]
    inp("wk", [L, DT, DT, P, P], FP32R)
    inp("wo", [L, DT, DT, P, P], FP32R)
    inp("wv", [L, DT, P, D], FP32R)       # [l, ktile, 128k, 768m]
    inp("w1", [L, FT, P, DT, P], FP32R)   # [l, mtile, 128k-part, ktile, 128m]
    inp("w2", [L, FT, P, D], FP32R)       # [l, ktile, 128k, 768m]
    inp("bq_c", [L, P, DT])
    inp("bk_c", [L, P, DT])
    inp("bv_r", [L, D])
    inp("bo_c", [L, P, DT])
    inp("b1_c", [L, P, FT])
    inp("b2_c", [L, P, DT])
    inp("g1_c", [L, P, DT])
    inp("g2_c", [L, P, DT])
    inp("gb1", [L, 2, D], FP32R)
    inp("gb2", [L, 2, D], FP32R)
    io["out"] = nc.declare_dram_parameter("out", [P, DT, T], FP32,
                                          isOutput=True)

    with tile.TileContext(nc) as tc:
        _emit(nc, tc, io)
    nc.compile()
    return nc


def _emit(nc, tc, io):
    from contextlib import ExitStack

    with ExitStack() as ctx:
        singles = ctx.enter_context(tc.tile_pool(name="singles", bufs=1))
        acts = ctx.enter_context(tc.tile_pool(name="acts", bufs=1))
        wpool = ctx.enter_context(tc.tile_pool(name="wpool", bufs=8))
        w1pool = ctx.enter_context(tc.tile_pool(name="w1pool", bufs=2))
        w2pool = ctx.enter_context(tc.tile_pool(name="w2pool", bufs=2))
        lw = ctx.enter_context(tc.tile_pool(name="lw", bufs=2))
        tmp = ctx.enter_context(tc.tile_pool(name="tmp", bufs=4))
        smalls = ctx.enter_context(tc.tile_pool(name="smalls", bufs=2))
        ps8 = ctx.enter_context(tc.tile_pool(name="ps8", bufs=8,
                                             space="PSUM"))

        # ---- persistent activations (feature-major unless noted) ----
        h = acts.tile([P, DT, T], FP32R)
        q = acts.tile([P, DT, T], FP32R)   # also holds attention output o
        k = acts.tile([P, DT, T], FP32R)
        v = acts.tile([P, TC, H, DN + 1], FP32R)  # token-major, +ones col
        o = q
        mm_sb = acts.tile([P, BL, SC, S], FP32)
        nc.sync.dma_start(mm_sb, io["mmask"][:])

        # ---- constants ----
        ident = singles.tile([P, P], FP32)
        make_identity(nc, ident)
        cst_f = singles.tile([P, 2], FP32)
        nc.vector.memset(cst_f[:, 0:1], 1.0)
        nc.vector.memset(cst_f[:, 1:2], 1.0 / D)
        cst_r = singles.tile([P, 2], FP32R)
        nc.vector.tensor_copy(cst_r, cst_f)
        ones_col = cst_r[:, 0:1]
        inv_d_col = cst_r[:, 1:2]
        row_f = singles.tile([1, CH], FP32)
        nc.vector.memset(row_f, 1.0)
        ones_row512 = singles.tile([1, CH], FP32R)
        nc.vector.tensor_copy(ones_row512, row_f)
        ones_row64 = ones_row512[:, 0:64]
        ones_row128 = ones_row512[:, 0:P]
        op64_f = singles.tile([65, 64], FP32)
        nc.vector.memset(op64_f[64:65, :], 1.0)
        ones_p64 = singles.tile([65, 64], FP32R)
        nc.vector.tensor_copy(ones_p64[64:65, :], op64_f[64:65, :])
        # ones column of v (written once; evictions only touch cols 0:DN)
        nc.vector.tensor_copy(v[:, :, :, DN],
                              cst_r[:, 0:1].to_broadcast((P, TC, H)))

        # ---- embedding gather + transpose to feature-major + positional ----
        with tc.tile_pool(name="embp", bufs=2) as embp:
            pe_sb = embp.tile([P, DT, S], FP32, bufs=1)
            nc.sync.dma_start(pe_sb, io["pe"][:])
            for c in range(TC):
                idx_t = embp.tile([P, 1], I32, tag="idx")
                nc.sync.dma_start(
                    idx_t, io["x_idx"][c].rearrange("(p o) -> p o", o=1))
                etok = embp.tile([P, D], FP32, tag="etok")
                nc.gpsimd.indirect_dma_start(
                    out=etok[:], out_offset=None, in_=io["emb"][:],
                    in_offset=bass.IndirectOffsetOnAxis(ap=idx_t[:, :1], axis=0))
                sc = c % SC  # position chunk within the sequence
                for ft in range(DT):
                    tp_ps = ps8.tile([P, P], FP32, tag="ps")
                    nc.tensor.transpose(tp_ps, etok[:, ft * P:(ft + 1) * P],
                                        ident)
                    nc.vector.tensor_tensor(
                        out=h[:, ft, c * P:(c + 1) * P], in0=tp_ps,
                        in1=pe_sb[:, ft, sc * P:(sc + 1) * P], op=OP.add)

        # ---- layers ----
        if REPS > 1:
            h0_save = nc.dram_tensor("h0_save", [P, DT, T], FP32R)
            nc.sync.dma_start(h0_save.ap(), h)
        for rep in range(REPS):
          if rep > 0:
            nc.sync.dma_start(h, h0_save.ap())
          for l in range(L):
            # per-layer bias / layernorm parameter tiles
            bq_t = lw.tile([P, DT], FP32, tag="bq")
            nc.sync.dma_start(bq_t, io["bq_c"][l])
            bk_t = lw.tile([P, DT], FP32, tag="bk")
            nc.sync.dma_start(bk_t, io["bk_c"][l])
            bo_t = lw.tile([P, DT], FP32, tag="bo")
            nc.sync.dma_start(bo_t, io["bo_c"][l])
            b1_t = lw.tile([P, FT], FP32, tag="b1")
            nc.sync.dma_start(b1_t, io["b1_c"][l])
            b2_t = lw.tile([P, DT], FP32, tag="b2")
            nc.sync.dma_start(b2_t, io["b2_c"][l])
            g1_t = lw.tile([P, DT], FP32, tag="g1")
            nc.sync.dma_start(g1_t, io["g1_c"][l])
            g2_t = lw.tile([P, DT], FP32, tag="g2")
            nc.sync.dma_start(g2_t, io["g2_c"][l])
            gneg1_t = lw.tile([1, D], FP32R, tag="gneg1", bufs=1)
            nc.sync.dma_start(gneg1_t, io["gb1"][l, 0:1, :])
            brow1_t = lw.tile([1, D], FP32R, tag="brow1", bufs=1)
            nc.sync.dma_start(brow1_t, io["gb1"][l, 1:2, :])
            gneg2_t = lw.tile([1, D], FP32R, tag="gneg2", bufs=1)
            nc.sync.dma_start(gneg2_t, io["gb2"][l, 0:1, :])
            brow2_t = lw.tile([1, D], FP32R, tag="brow2", bufs=1)
            nc.sync.dma_start(brow2_t, io["gb2"][l, 1:2, :])
            gb1_t = (gneg1_t, brow1_t, ones_row512)
            gb2_t = (gneg2_t, brow2_t, ones_row512)
            bv_t = lw.tile([P, D], FP32, tag="bv", bufs=1)
            bvl = io["bv_r"][l]
            nc.sync.dma_start(
                bv_t, bass.AP(tensor=bvl.tensor, offset=bvl.offset,
                              ap=[[0, P]] + list(bvl.ap)))

            # ---------- q/k projections ([P,P] weight blocks) ----------
            if "qkv" not in SKIP:
                for wname, bias_t, dst in (("wq", bq_t, q), ("wk", bk_t, k)):
                    for mt in range(DT):
                        for ch in range(NCH):
                            ps = ps8.tile([P, CH], FP32, tag="ps")
                            for kt in range(DT):
                                wt = wpool.tile([P, P], FP32R, tag="wblk")
                                if NODMAW:
                                    nc.sync.dma_start(wt[:, 0:1],
                                                      io[wname][l, kt, mt][:, 0:1])
                                else:
                                    nc.sync.dma_start(wt, io[wname][l, kt, mt])
                                nc.tensor.matmul(
                                    ps, lhsT=wt,
                                    rhs=h[:, kt, ch * CH:(ch + 1) * CH],
                                    start=(kt == 0), stop=(kt == DT - 1))
                            nc.vector.tensor_scalar(
                                out=dst[:, mt, ch * CH:(ch + 1) * CH], in0=ps,
                                scalar1=bias_t[:, mt:mt + 1], scalar2=None,
                                op0=OP.add)
                # v (token-major): out[t_chunk, features], ktile weights
                with tc.tile_pool(name="wvp", bufs=7) as wvp:
                    wts = []
                    for kt in range(DT):
                        wt = wvp.tile([P, D], FP32R, tag="wv")
                        if NODMAW:
                            nc.sync.dma_start(wt[:, 0:1], io["wv"][l, kt][:, 0:1])
                        else:
                            nc.sync.dma_start(wt, io["wv"][l, kt])
                        wts.append(wt)
                    HD = D // 2
                    for tch in range(TC):
                        for nh in range(2):
                            ps = ps8.tile([P, CH], FP32, tag="ps")
                            psn = ps[:, :HD]
                            for kt in range(DT):
                                nc.tensor.matmul(
                                    psn, lhsT=h[:, kt, tch * P:(tch + 1) * P],
                                    rhs=wts[kt][:, nh * HD:(nh + 1) * HD],
                                    start=(kt == 0), stop=(kt == DT - 1))
                            nc.vector.tensor_tensor(
                                out=v[:, tch, nh * (H // 2):(nh + 1) * (H // 2),
                                      0:DN],
                                in0=psn.rearrange("p (hh e) -> p hh e", e=DN),
                                in1=bv_t[:, nh * HD:(nh + 1) * HD].rearrange(
                                    "p (hh e) -> p hh e", e=DN),
                                op=OP.add)

            # ---------- attention ----------
            if "att" in SKIP:
                attp = None
            else:
             with tc.tile_pool(name="attp", bufs=3) as attp:
                for bb in range(BL):
                    for hp in range(DT):  # head pair: heads 2hp, 2hp+1
                        exs = []
                        for hh in range(2):
                            ex = attp.tile([P, SC, S], FP32R, tag="ex")
                            pr = slice(hh * 64, (hh + 1) * 64)
                            for tci in range(SC):
                                st = ps8.tile([P, S], FP32, tag="ps")
                                nc.tensor.matmul(
                                    st,
                                    lhsT=k[pr, hp, bb * S + tci * P:
                                           bb * S + (tci + 1) * P],
                                    rhs=q[pr, hp, bb * S:(bb + 1) * S],
                                    start=True, stop=True)
                                nc.scalar.activation(ex[:, tci, :], st,
                                                     AF.Exp)
                            eng = nc.vector if hh == 0 else nc.gpsimd
                            eng.tensor_tensor(out=ex[:], in0=ex[:],
                                              in1=mm_sb[:, bb], op=OP.mult)
                            exs.append(ex)
                        pvs = [ps8.tile([65, S], FP32, tag="ps",
                                           name=f"pv{i}") for i in range(2)]
                        for tci in range(SC):
                            tg = bb * SC + tci
                            for hh in range(2):
                                hd = 2 * hp + hh
                                nc.tensor.matmul(
                                    pvs[hh],
                                    lhsT=v[:, tg, hd, :],
                                    rhs=exs[hh][:, tci, :],
                                    start=(tci == 0), stop=(tci == SC - 1))
                        rc = smalls.tile([65, 2, S], FP32R, tag="rc")
                        with nc.allow_low_precision(
                                reason="fp32r softmax denominators"):
                            nc.vector.reciprocal(rc[64:65, 0, :],
                                                 pvs[0][64:65, :])
                            nc.vector.reciprocal(rc[64:65, 1, :],
                                                 pvs[1][64:65, :])
                        bcs = [ps8.tile([64, S], FP32, tag="ps",
                                           name=f"bc{i}") for i in range(2)]
                        nc.tensor.matmul(bcs[0], lhsT=ones_p64[64:65, :],
                                         rhs=rc[64:65, 0, :], start=True,
                                         stop=True)
                        nc.tensor.matmul(bcs[1], lhsT=ones_p64[64:65, :],
                                         rhs=rc[64:65, 1, :], start=True,
                                         stop=True)
                        # head even: normalize straight into o[0:64]
                        nc.vector.tensor_copy(
                            o[0:64, hp, bb * S:(bb + 1) * S], pvs[0][0:64, :])
                        nc.vector.tensor_tensor(
                            out=o[0:64, hp, bb * S:(bb + 1) * S],
                            in0=o[0:64, hp, bb * S:(bb + 1) * S], in1=bcs[0],
                            op=OP.mult)
                        # head odd: normalize at partitions 0-63, then
                        # DMA-shift into partitions 64-127 of o
                        ot = tmp.tile([64, S], FP32R, tag="scr")
                        nc.vector.tensor_copy(ot, pvs[1][0:64, :])
                        nc.vector.tensor_tensor(out=ot, in0=ot, in1=bcs[1],
                                                op=OP.mult)
                        nc.sync.dma_start(
                            o[64:128, hp, bb * S:(bb + 1) * S], ot)

            # ---------- Wo + residual, then LN1 ----------
            if "wo" not in SKIP:
                for mt in range(DT):
                    for ch in range(NCH):
                        ps = ps8.tile([P, CH], FP32, tag="ps")
                        for kt in range(DT):
                            wt = wpool.tile([P, P], FP32R, tag="wblk")
                            if NODMAW:
                                nc.sync.dma_start(wt[:, 0:1],
                                                  io["wo"][l, kt, mt][:, 0:1])
                            else:
                                nc.sync.dma_start(wt, io["wo"][l, kt, mt])
                            nc.tensor.matmul(
                                ps, lhsT=wt,
                                rhs=o[:, kt, ch * CH:(ch + 1) * CH],
                                start=(kt == 0), stop=(kt == DT - 1))
                        nc.vector.scalar_tensor_tensor(
                            out=h[:, mt, ch * CH:(ch + 1) * CH], in0=ps,
                            scalar=bo_t[:, mt:mt + 1],
                            in1=h[:, mt, ch * CH:(ch + 1) * CH],
                            op0=OP.add, op1=OP.add)
            if "ln" not in SKIP:
                _layernorm(nc, tc, h, g1_t, gb1_t, ones_row128, inv_d_col,
                           tmp, smalls, ps8)

            # ---------- FFN + residual, then LN2 ----------
            if "ffn" not in SKIP:
                for ch in range(NCH):
                    accs = [ps8.tile([P, CH], FP32, tag="ps",
                                        name=f"acc{i}")
                            for i in range(DT)]
                    for m in range(FT):
                        w1t = w1pool.tile([P, DT, P], FP32R, tag="w1")
                        w2t = w2pool.tile([P, D], FP32R, tag="w2")
                        if NODMAW:
                            nc.sync.dma_start(w1t[:, :, 0:1],
                                              io["w1"][l, m][:, :, 0:1])
                            nc.sync.dma_start(w2t[:, 0:1],
                                              io["w2"][l, m][:, 0:1])
                        else:
                            nc.sync.dma_start(w1t, io["w1"][l, m])
                            nc.sync.dma_start(w2t, io["w2"][l, m])
                        ps = ps8.tile([P, CH], FP32, tag="ps")
                        for kt in range(DT):
                            nc.tensor.matmul(
                                ps, lhsT=w1t[:, kt, :],
                                rhs=h[:, kt, ch * CH:(ch + 1) * CH],
                                start=(kt == 0), stop=(kt == DT - 1))
                        ff_sb = tmp.tile([P, CH], FP32R, tag="scr")
                        nc.scalar.activation(ff_sb, ps, AF.Relu,
                                             bias=b1_t[:, m:m + 1])
                        for mt in range(DT):
                            nc.tensor.matmul(
                                accs[mt], lhsT=w2t[:, mt * P:(mt + 1) * P],
                                rhs=ff_sb, start=(m == 0), stop=(m == FT - 1))
                    for mt in range(DT):
                        nc.vector.scalar_tensor_tensor(
                            out=h[:, mt, ch * CH:(ch + 1) * CH], in0=accs[mt],
                            scalar=b2_t[:, mt:mt + 1],
                            in1=h[:, mt, ch * CH:(ch + 1) * CH],
                            op0=OP.add, op1=OP.add)
            if "ln" not in SKIP:
                _layernorm(nc, tc, h, g2_t, gb2_t, ones_row128, inv_d_col,
                           tmp, smalls, ps8)

        nc.sync.dma_start(io["out"][:], h[:].bitcast(FP32))


def _layernorm(nc, tc, h, g_t, gb_t, ones_row128, inv_d_col, tmp, smalls,
               ps8):
    """In-place LayerNorm over the feature (partition) axis of h [P, DT, T]."""
    if True:
        for ch in range(NCH):
            chs = slice(ch * CH, (ch + 1) * CH)
            mean_ps = ps8.tile([P, CH], FP32, tag="ps")
            msq_ps = ps8.tile([P, CH], FP32, tag="ps")
            for mt in range(DT):
                sq = tmp.tile([P, CH], FP32R, tag="scr")
                nc.scalar.activation(sq, h[:, mt, chs], AF.Square)
                nc.tensor.matmul(mean_ps[0:1, :], lhsT=inv_d_col,
                                 rhs=h[:, mt, chs], start=(mt == 0),
                                 stop=(mt == DT - 1))
                nc.tensor.matmul(msq_ps[0:1, :], lhsT=inv_d_col, rhs=sq,
                                 start=(mt == 0), stop=(mt == DT - 1))
            sqm = smalls.tile([1, CH], FP32, tag="s")
            nc.scalar.activation(sqm, mean_ps[0:1, :], AF.Square)
            var = smalls.tile([1, CH], FP32, tag="s")
            nc.vector.scalar_tensor_tensor(out=var, in0=msq_ps[0:1, :],
                                           scalar=EPS, in1=sqm, op0=OP.add,
                                           op1=OP.subtract)
            lnv = smalls.tile([1, CH], FP32, tag="s")
            nc.scalar.activation(lnv, var, AF.Ln)
            mr = smalls.tile([1, CH], FP32R, tag="s")
            nc.scalar.activation(mr, lnv, AF.Exp, scale=-0.5)  # rstd
            mmr = smalls.tile([1, CH], FP32R, tag="s")
            nc.vector.tensor_tensor(out=mmr, in0=mean_ps[0:1, :], in1=mr,
                                    op=OP.mult)            # mean*rstd
            rstd_b = ps8.tile([P, CH], FP32, tag="ps")
            nc.tensor.matmul(rstd_b, lhsT=ones_row128, rhs=mr,
                             start=True, stop=True)
            gneg_t, brow_t, ones_row512 = gb_t
            for mt in range(DT):
                c2 = ps8.tile([P, CH], FP32, tag="ps")
                nc.tensor.matmul(c2, lhsT=gneg_t[:, mt * P:(mt + 1) * P],
                                 rhs=mmr, start=True, stop=False)
                nc.tensor.matmul(c2, lhsT=brow_t[:, mt * P:(mt + 1) * P],
                                 rhs=ones_row512, start=False, stop=True)
                t2 = tmp.tile([P, CH], FP32, tag="scr")
                nc.vector.tensor_tensor(out=t2, in0=h[:, mt, chs], in1=rstd_b,
                                        op=OP.mult)
                nc.vector.scalar_tensor_tensor(
                    out=h[:, mt, chs], in0=t2, scalar=g_t[:, mt:mt + 1],
                    in1=c2, op0=OP.mult, op1=OP.add)


# ---------------- host side ----------------

def _pos_encoding_np():
    pos = np.arange(S, dtype=np.float32)[:, None]
    i = np.arange(D // 2, dtype=np.float32)[None, :]
    denom_s = np.power(np.float32(10000.0), (2.0 * i / D).astype(np.float32))
    denom_c = np.power(np.float32(10000.0),
                       (2.0 * (i + 1.0) / D).astype(np.float32))
    pe = np.zeros((S, D), np.float32)
    pe[:, 0::2] = np.sin(pos / denom_s)
    pe[:, 1::2] = np.cos(pos / denom_c)
    return pe  # [S, D]


def _prep_shared(emb, Wq, bq, Wk, bk, Wv, bv, Wo, bo, W1, b1, W2, b2,
                 g1, be1, g2, be2):
    f32 = np.float32
    scale = f32(1.0 / np.sqrt(DN))

    def cols(a, nt):  # [L, nt*128] -> [L, 128, nt]
        return np.ascontiguousarray(
            np.asarray(a).reshape(L, nt, P).transpose(0, 2, 1)).astype(f32)

    def blocks(a):  # [L, D, D] -> [L, DT, DT, P, P] (ktile, mtile blocks)
        return np.ascontiguousarray(
            a.reshape(L, DT, P, DT, P).transpose(0, 1, 3, 2, 4)).astype(f32)

    Wq, Wk, Wv, Wo = (np.asarray(a)[:L] for a in (Wq, Wk, Wv, Wo))
    W1, W2 = np.asarray(W1)[:L], np.asarray(W2)[:L]
    bq, bk, bv, bo = (np.asarray(a)[:L] for a in (bq, bk, bv, bo))
    b1, b2 = np.asarray(b1)[:L], np.asarray(b2)[:L]
    g1, be1, g2, be2 = (np.asarray(a)[:L] for a in (g1, be1, g2, be2))

    wq_h = blocks(Wq.transpose(0, 2, 1, 3).reshape(L, D, D) * scale)
    wk_h = blocks(Wk.transpose(0, 2, 1, 3).reshape(L, D, D))
    wo_h = blocks(Wo.astype(f32))
    wv_h = np.ascontiguousarray(
        Wv.transpose(0, 2, 1, 3).reshape(L, DT, P, D)).astype(f32)
    w1_h = np.ascontiguousarray(
        W1.reshape(L, DT, P, FT, P).transpose(0, 3, 2, 1, 4)).astype(f32)
    w2_h = np.ascontiguousarray(W2.reshape(L, FT, P, D)).astype(f32)

    pe_np = _pos_encoding_np()  # [S, D]
    pe_h = np.ascontiguousarray(
        pe_np.T.reshape(DT, P, S).transpose(1, 0, 2)).astype(f32)

    return dict(
        emb=np.ascontiguousarray(emb).astype(f32),
        pe=pe_h,
        wq=wq_h, wk=wk_h, wv=wv_h, wo=wo_h, w1=w1_h, w2=w2_h,
        bq_c=cols(bq.reshape(L, D) * scale, DT),
        bk_c=cols(bk.reshape(L, D), DT),
        bv_r=np.ascontiguousarray(bv.reshape(L, D)).astype(f32),
        bo_c=cols(bo, DT),
        b1_c=cols(b1, FT),
        b2_c=cols(b2, DT),
        g1_c=cols(g1, DT),
        g2_c=cols(g2, DT),
        gb1=np.ascontiguousarray(np.stack([-g1, be1], axis=1)).astype(f32),
        gb2=np.ascontiguousarray(np.stack([-g2, be2], axis=1)).astype(f32),
    )


def kernel(x, padding_mask, emb, Wq, bq, Wk, bk, Wv, bv, Wo, bo,
           W1, b1, W2, b2, g1, be1, g2, be2):
    if "nc" not in _PROGRAM_CACHE:
        _PROGRAM_CACHE["nc"] = _build_program()
    nc = _PROGRAM_CACHE["nc"]

    shared = _prep_shared(emb, Wq, bq, Wk, bk, Wv, bv, Wo, bo, W1, b1, W2, b2,
                          g1, be1, g2, be2)

    x_i = np.asarray(x).astype(np.int32)
    mask_f = 1.0 - np.asarray(padding_mask).astype(np.float32)

    in_maps = []
    for c in range(NCORES):
        xs = x_i[c * BL:(c + 1) * BL].reshape(T)             # [1024]
        ms = mask_f[c * BL:(c + 1) * BL]                     # [2, 512, 512]
        # mmask[p, b, tci, s] = (1-mask)[b, s, tci*128+p]
        mt = np.ascontiguousarray(
            ms.transpose(0, 2, 1).reshape(BL, SC, P, S).transpose(2, 0, 1, 3))
        m = dict(shared)
        m["x_idx"] = np.ascontiguousarray(xs.reshape(TC, P))
        m["mmask"] = mt
        in_maps.append(m)

    res = run_bass_kernel_spmd(nc, in_maps, core_ids=list(range(NCORES)))

    outs = []
    for c in range(NCORES):
        oc = res.results[c]["out"]                    # [P, DT, T]
        hc = oc.transpose(2, 1, 0).reshape(T, D)      # [T, D]
        outs.append(hc.reshape(BL, S, D))
    return np.concatenate(outs, axis=0).astype(np.float32)


if __name__ == "__main__":
    pass



# revision 13
# speedup vs baseline: 30.1474x; 30.1474x over previous
"""Trainium2 Bass kernel for nn_Encoder: 6-layer post-LN transformer encoder.

Sharding: pure data-parallel over batch across 8 NeuronCores (2 sequences per
core), zero collectives. On-device layout is feature-major ([D on partitions,
tokens on free dim]) so every projection uses the stored weight directly as the
matmul stationary operand and per-feature biases are per-partition scalars.

The embedding gather + positional encoding happen on the host (3 MB of
activations per core instead of a 98 MB table). All matmul operands (weights
and activations) are bf16 with fp32 PSUM accumulation; the residual stream and
LayerNorm stay fp32. Weight tensors are laid out so each layer's projection
weights arrive in one 9 KB-per-partition-line DMA.

The two sequences a core owns are independent streams (token chunks ch=0/1);
per-stage work is emitted chunk-interleaved so one stream's matmuls hide the
other stream's softmax/LayerNorm latency chains.

Attention computes transposed scores [t, s] per head as two concurrent K=64
row-tiled matmuls (partition halves of q/k), exponentiates without
max-subtraction (scores are O(1) by construction; masking is exp(s)*(1-m),
exact since exp(-1e9) underflows to 0), and contracts PV with M=66 matmuls:
rows 0-63 the head output, row 64/65 the softmax denominator for the
even/odd head (from ones-columns in V). One K=2 matmul broadcasts both
reciprocal denominators across the partition halves for normalization.

LayerNorm reduces over the feature (partition) axis with ones-matmuls
(E[x^2]-E[x]^2+eps), then applies (z*rstd)*g + (-g*mean*rstd + b) where the
per-token row factors are broadcast across partitions via K=1/K=2 PE outer
products and per-feature factors are per-partition scalars.
"""

import os
import sys

import numpy as np

for _p in ("/root/.axon_site/_ro/trn_rl_repo", "/opt/trn_rl_repo"):
    if os.path.isdir(_p) and _p not in sys.path:
        sys.path.append(_p)

import concourse.bass as bass  # noqa: E402
import concourse.mybir as mybir  # noqa: E402
import concourse.tile as tile  # noqa: E402
from concourse import bacc  # noqa: E402
from concourse.bass_utils import run_bass_kernel_spmd  # noqa: E402

# Problem constants (hardcoded per harness contract).
V, D, H, F = 32000, 768, 12, 3072
L = int(os.environ.get("ENC_LAYERS", "6"))
DN = D // H            # 64
B, S = 16, 512
NCORES = 8
BL = B // NCORES       # 2 sequences per core
T = BL * S             # 1024 tokens per core
P = 128
DT = D // P            # 6 feature tiles
TC = T // P            # 8 token chunks
SC = S // P            # 4 chunks per sequence
FT = F // P            # 24 ff tiles
NCH = 2                # T split into chunks of 512 (= one sequence each)
CH = T // NCH          # 512
EPS = 1e-5
FP32 = mybir.dt.float32
FP32R = mybir.dt.float32r
BF16 = mybir.dt.bfloat16
FP8 = mybir.dt.float8e4
I32 = mybir.dt.int32

AF = mybir.ActivationFunctionType
OP = mybir.AluOpType

_PROGRAM_CACHE = {}


def _build_program(reps=None):
    """Build + compile the per-core program. reps>1 reruns the whole encoder
    (h reloaded from DRAM each rep) for marginal-time measurement."""
    if reps is None:
        reps = int(os.environ.get("ENC_REPS", "1"))
    nc = bacc.Bacc("TRN2", target_bir_lowering=False, debug=False,
                   num_devices=NCORES)

    io = {}

    def inp(name, shape, dtype=FP32):
        io[name] = nc.declare_dram_parameter(name, list(shape), dtype,
                                             isOutput=False)

    inp("h0", [P, DT, T], FP32R)          # host: (emb[x]+pe), feature-major
    inp("mmask", [P, BL, SC, S], FP8)     # host: 1 - padding_mask, transposed
    inp("wq", [L, P, DT, DT, P], BF16)    # [l, 128k, mtile, ktile, 128m]
    inp("wk", [L, P, DT, DT, P], BF16)
    inp("wo", [L, P, DT, DT, P], BF16)
    inp("wv", [L, P, DT, D], BF16)        # [l, 128k, ktile, 768m]
    inp("w1", [L, FT, P, DT, P], BF16)    # [l, mtile, 128k, ktile, 128m]
    inp("w2", [L, DT, P, FT, P], BF16)    # [l, mtile, 128k, ktile, 128m]
    inp("bq_c", [L, P, DT])
    inp("bk_c", [L, P, DT])
    inp("bv_r", [L, D])
    inp("bo_c", [L, P, DT])
    inp("b1_c", [L, P, FT])
    inp("b2_c", [L, P, DT])
    inp("g1_c", [L, P, DT])
    inp("g2_c", [L, P, DT])
    inp("gb1", [L, 2, D], FP32R)
    inp("gb2", [L, 2, D], FP32R)
    io["out"] = nc.declare_dram_parameter("out", [P, DT, T], FP32,
                                          isOutput=True)

    with tile.TileContext(nc) as tc:
        _emit(nc, tc, io, reps)
    nc.compile()
    return nc


def _emit(nc, tc, io, reps):
    from contextlib import ExitStack

    with ExitStack() as ctx:
        singles = ctx.enter_context(tc.tile_pool(name="singles", bufs=1))
        acts = ctx.enter_context(tc.tile_pool(name="acts", bufs=1))
        wproj = ctx.enter_context(tc.tile_pool(name="wproj", bufs=1))
        wpool = ctx.enter_context(tc.tile_pool(name="wpool", bufs=4))
        w1pool = ctx.enter_context(tc.tile_pool(name="w1pool", bufs=3))
        w2pool = ctx.enter_context(tc.tile_pool(name="w2pool", bufs=2))
        ffp = ctx.enter_context(tc.tile_pool(name="ffp", bufs=1))
        lw = ctx.enter_context(tc.tile_pool(name="lw", bufs=2))
        tmp = ctx.enter_context(tc.tile_pool(name="tmp", bufs=2))
        smalls = ctx.enter_context(tc.tile_pool(name="smalls", bufs=2))
        ps8 = ctx.enter_context(tc.tile_pool(name="ps8", bufs=8,
                                             space="PSUM"))

        # ---- persistent activations (feature-major unless noted) ----
        h = acts.tile([P, DT, T], FP32R)
        hb = acts.tile([P, DT, T], BF16)   # bf16 copy of h (matmul rhs)
        q = acts.tile([P, DT, T], BF16)    # also holds attention output o
        k = acts.tile([P, DT, T], BF16)
        v = acts.tile([P, TC, H, DN + 1], BF16)  # token-major, +ones col
        o = q
        mm_sb = acts.tile([P, BL, SC, S], FP8)
        nc.sync.dma_start(mm_sb, io["mmask"][:])

        # ---- constants ----
        cst_f = singles.tile([P, 2], FP32)
        nc.vector.memset(cst_f[:, 0:1], 1.0)
        nc.vector.memset(cst_f[:, 1:2], 1.0 / D)
        cst_r = singles.tile([P, 2], FP32R)
        nc.vector.tensor_copy(cst_r, cst_f)
        inv_d_col = cst_r[:, 1:2]
        row2_f = singles.tile([2, CH], FP32)
        nc.vector.memset(row2_f, 1.0)
        ones2 = singles.tile([2, CH], FP32R)
        nc.vector.tensor_copy(ones2, row2_f)
        ones_row128 = ones2[0:1, 0:P]
        op64_f = singles.tile([65, 64], FP32)
        nc.vector.memset(op64_f[64:65, :], 1.0)
        ones_p64 = singles.tile([65, 64], FP32R)
        nc.vector.tensor_copy(ones_p64[64:65, :], op64_f[64:65, :])
        # ones column of v (written once; evictions only touch cols 0:DN)
        nc.vector.tensor_copy(v[:, :, :, DN],
                              cst_r[:, 0:1].to_broadcast((P, TC, H)))

        ln_env = (ones_row128, ones2, inv_d_col, tmp, smalls, ps8)

        for rep in range(reps):
          nc.sync.dma_start(h, io["h0"][:])
          for mt in range(DT):
              nc.gpsimd.tensor_copy(hb[:, mt, :], h[:, mt, :])
          for l in range(L):
            # per-layer bias / layernorm parameter tiles
            bq_t = lw.tile([P, DT], FP32, tag="bq")
            nc.sync.dma_start(bq_t, io["bq_c"][l])
            bk_t = lw.tile([P, DT], FP32, tag="bk")
            nc.sync.dma_start(bk_t, io["bk_c"][l])
            bo_t = lw.tile([P, DT], FP32, tag="bo")
            nc.sync.dma_start(bo_t, io["bo_c"][l])
            b1_t = lw.tile([P, FT], FP32, tag="b1")
            nc.sync.dma_start(b1_t, io["b1_c"][l])
            b2_t = lw.tile([P, DT], FP32, tag="b2")
            nc.sync.dma_start(b2_t, io["b2_c"][l])
            g1_t = lw.tile([P, DT], FP32, tag="g1")
            nc.sync.dma_start(g1_t, io["g1_c"][l])
            g2_t = lw.tile([P, DT], FP32, tag="g2")
            nc.sync.dma_start(g2_t, io["g2_c"][l])
            gb1_t = lw.tile([2, D], FP32R, tag="gb1", bufs=1)
            nc.sync.dma_start(gb1_t, io["gb1"][l])
            gb2_t = lw.tile([2, D], FP32R, tag="gb2", bufs=1)
            nc.sync.dma_start(gb2_t, io["gb2"][l])
            bv_t = lw.tile([P, D], FP32, tag="bv", bufs=1)
            bvl = io["bv_r"][l]
            nc.sync.dma_start(
                bv_t, bass.AP(tensor=bvl.tensor, offset=bvl.offset,
                              ap=[[0, P]] + list(bvl.ap)))

            # per-layer wo/wv weights: one big DMA each (>=1.5KB lines)
            wo_t = wproj.tile([P, DT, DT, P], BF16, tag="wo")
            nc.sync.dma_start(wo_t, io["wo"][l])
            wv_t = wproj.tile([P, DT, D], BF16, tag="wv")
            nc.sync.dma_start(wv_t, io["wv"][l])

            # ---------- q/k projections (weights streamed per m-tile) ----
            for wname, bias_t, dst in (("wq", bq_t, q), ("wk", bk_t, k)):
                for mt in range(DT):
                    wt = wpool.tile([P, DT, P], BF16, tag="wblk")
                    nc.sync.dma_start(wt, io[wname][l][:, mt])
                    for ch in range(NCH):
                        ps = ps8.tile([P, CH], FP32, tag="ps")
                        for kt in range(DT):
                            nc.tensor.matmul(
                                ps, lhsT=wt[:, kt, :],
                                rhs=hb[:, kt, ch * CH:(ch + 1) * CH],
                                start=(kt == 0), stop=(kt == DT - 1))
                        nc.vector.tensor_scalar(
                            out=dst[:, mt, ch * CH:(ch + 1) * CH], in0=ps,
                            scalar1=bias_t[:, mt:mt + 1], scalar2=None,
                            op0=OP.add)
            # v (token-major): out[t_chunk, features], ktile weights
            HD = D // 2
            for tch in range(TC):
                for nh in range(2):
                    ps = ps8.tile([P, CH], FP32, tag="ps")
                    psn = ps[:, :HD]
                    for kt in range(DT):
                        nc.tensor.matmul(
                            psn, lhsT=hb[:, kt, tch * P:(tch + 1) * P],
                            rhs=wv_t[:, kt, nh * HD:(nh + 1) * HD],
                            start=(kt == 0), stop=(kt == DT - 1))
                    nc.vector.tensor_tensor(
                        out=v[:, tch, nh * (H // 2):(nh + 1) * (H // 2),
                              0:DN],
                        in0=psn.rearrange("p (hh e) -> p hh e", e=DN),
                        in1=bv_t[:, nh * HD:(nh + 1) * HD].rearrange(
                            "p (hh e) -> p hh e", e=DN),
                        op=OP.add)

            # ---------- attention ----------
            # software-pipelined over head pairs: scores/exp/mask of pair
            # i+1 are emitted before PV/normalize of pair i so the PE has
            # score matmuls to run while pair i's softmax chain completes.
            with tc.tile_pool(name="attp", bufs=6) as attp:
                pend = []

                def produce(bb, hp):
                    exs = [attp.tile([P, SC, S], BF16, tag="ex",
                                     name=f"ex{i}") for i in range(2)]
                    for tci in range(SC):
                        sts = []
                        for hh in range(2):
                            pr = slice(hh * 64, (hh + 1) * 64)
                            st = ps8.tile([P, S], FP32, tag="ps",
                                          name=f"st{hh}")
                            nc.tensor.matmul(
                                st,
                                lhsT=k[pr, hp, bb * S + tci * P:
                                       bb * S + (tci + 1) * P],
                                rhs=q[pr, hp, bb * S:(bb + 1) * S],
                                start=True, stop=True,
                                tile_position=(hh * 64, 0))
                            sts.append(st)
                        for hh in range(2):
                            nc.scalar.activation(exs[hh][:, tci, :],
                                                 sts[hh], AF.Exp)
                            eng = nc.vector if hh == 0 else nc.gpsimd
                            eng.tensor_tensor(
                                out=exs[hh][:, tci, :],
                                in0=exs[hh][:, tci, :],
                                in1=mm_sb[:, bb, tci, :], op=OP.mult)
                    return exs

                def consume(bb, hp, exs):
                    pvs = [ps8.tile([65, S], FP32, tag="ps",
                                       name=f"pv{i}") for i in range(2)]
                    for tci in range(SC):
                        tg = bb * SC + tci
                        for hh in range(2):
                            hd = 2 * hp + hh
                            nc.tensor.matmul(
                                pvs[hh],
                                lhsT=v[:, tg, hd, :],
                                rhs=exs[hh][:, tci, :],
                                start=(tci == 0), stop=(tci == SC - 1))
                    rcs = [smalls.tile([65, S], FP32R, tag="rc",
                                       name=f"rc{i}") for i in range(2)]
                    with nc.allow_low_precision(
                            reason="fp32r softmax denominators"):
                        nc.vector.reciprocal(rcs[0][64:65, :],
                                             pvs[0][64:65, :])
                        nc.vector.reciprocal(rcs[1][64:65, :],
                                             pvs[1][64:65, :])
                    bcs = [ps8.tile([64, S], FP32, tag="ps",
                                       name=f"bc{i}") for i in range(2)]
                    nc.tensor.matmul(bcs[0], lhsT=ones_p64[64:65, :],
                                     rhs=rcs[0][64:65, :], start=True,
                                     stop=True)
                    nc.tensor.matmul(bcs[1], lhsT=ones_p64[64:65, :],
                                     rhs=rcs[1][64:65, :], start=True,
                                     stop=True)
                    # head even: normalize straight into o[0:64]
                    nc.vector.tensor_copy(
                        o[0:64, hp, bb * S:(bb + 1) * S], pvs[0][0:64, :])
                    nc.vector.tensor_tensor(
                        out=o[0:64, hp, bb * S:(bb + 1) * S],
                        in0=o[0:64, hp, bb * S:(bb + 1) * S],
                        in1=bcs[0], op=OP.mult)
                    # head odd: normalize at partitions 0-63, then
                    # DMA-shift into partitions 64-127 of o
                    ot = tmp.tile([64, S], BF16, tag="ot")
                    nc.scalar.activation(ot, pvs[1][0:64, :], AF.Copy)
                    nc.vector.tensor_tensor(out=ot, in0=ot, in1=bcs[1],
                                            op=OP.mult)
                    nc.sync.dma_start(
                        o[64:128, hp, bb * S:(bb + 1) * S], ot)

                for bb in range(BL):
                    for hp in range(DT):  # head pair: heads 2hp, 2hp+1
                        exs = produce(bb, hp)
                        if len(pend) >= 2:
                            consume(*pend.pop(0))
                        pend.append((bb, hp, exs))
                while pend:
                    consume(*pend.pop(0))

            # ---------- Wo + residual + LN1, FFN + residual + LN2 ----
            # Each LayerNorm is split into stats (PE reduction) and finish
            # (scalar chain + apply); finishes are emitted after the other
            # chunk's matmul stage so the chains hide under PE work.
            st1 = []
            for ch in range(NCH):
                for mt in range(DT):
                    ps = ps8.tile([P, CH], FP32, tag="ps")
                    for kt in range(DT):
                        nc.tensor.matmul(
                            ps, lhsT=wo_t[:, mt, kt, :],
                            rhs=o[:, kt, ch * CH:(ch + 1) * CH],
                            start=(kt == 0), stop=(kt == DT - 1))
                    nc.vector.scalar_tensor_tensor(
                        out=h[:, mt, ch * CH:(ch + 1) * CH], in0=ps,
                        scalar=bo_t[:, mt:mt + 1],
                        in1=h[:, mt, ch * CH:(ch + 1) * CH],
                        op0=OP.add, op1=OP.add)
                st1.append(_ln_stats(nc, h, ch, ln_env))
            for ch in range(NCH):
                _ln_finish(nc, h, hb, ch, g1_t, gb1_t, st1[ch], ln_env)

            st2 = []
            for ch in range(NCH):
                # phase 1: ff = relu((h W1 + b1) / FSC), stored fp8 in SBUF
                ff_all = ffp.tile([P, FT, CH], BF16, tag="ffall")
                for m in range(FT):
                    w1t = w1pool.tile([P, DT, P], BF16, tag="w1")
                    nc.sync.dma_start(w1t, io["w1"][l, m])
                    ps = ps8.tile([P, CH], FP32, tag="ps")
                    for kt in range(DT):
                        nc.tensor.matmul(
                            ps, lhsT=w1t[:, kt, :],
                            rhs=hb[:, kt, ch * CH:(ch + 1) * CH],
                            start=(kt == 0), stop=(kt == DT - 1))
                    nc.scalar.activation(ff_all[:, m, :], ps, AF.Relu,
                                         bias=b1_t[:, m:m + 1])
                # phase 2: W2-major accumulation, one PSUM bank per out tile
                for mt in range(DT):
                    w2m = w2pool.tile([P, FT, P], BF16, tag="w2")
                    nc.sync.dma_start(w2m, io["w2"][l, mt])
                    acc = ps8.tile([P, CH], FP32, tag="ps", name="acc")
                    for m in range(FT):
                        nc.tensor.matmul(
                            acc, lhsT=w2m[:, m, :],
                            rhs=ff_all[:, m, :],
                            start=(m == 0), stop=(m == FT - 1))
                    nc.vector.scalar_tensor_tensor(
                        out=h[:, mt, ch * CH:(ch + 1) * CH], in0=acc,
                        scalar=b2_t[:, mt:mt + 1],
                        in1=h[:, mt, ch * CH:(ch + 1) * CH],
                        op0=OP.add, op1=OP.add)
                st2.append(_ln_stats(nc, h, ch, ln_env))
            for ch in range(NCH):
                _ln_finish(nc, h, hb if l < L - 1 else None, ch, g2_t,
                           gb2_t, st2[ch], ln_env)

        nc.sync.dma_start(io["out"][:], h[:].bitcast(FP32))


def _ln_stats(nc, h, ch, ln_env):
    """Phase-1 LayerNorm: per-token mean and mean-square via PE reduction."""
    ones_row128, ones2, inv_d_col, tmp, smalls, ps8 = ln_env
    chs = slice(ch * CH, (ch + 1) * CH)
    mean_ps = ps8.tile([1, CH], FP32, tag="ps", name="mean_ps")
    msq_ps = ps8.tile([1, CH], FP32, tag="ps", name="msq_ps")
    for mt in range(DT):
        sq = tmp.tile([P, CH], FP32R, tag="sq")
        nc.scalar.activation(sq, h[:, mt, chs], AF.Square)
        nc.tensor.matmul(mean_ps[0:1, :], lhsT=inv_d_col,
                         rhs=h[:, mt, chs], start=(mt == 0),
                         stop=(mt == DT - 1))
        nc.tensor.matmul(msq_ps[0:1, :], lhsT=inv_d_col, rhs=sq,
                         start=(mt == 0), stop=(mt == DT - 1))
    return mean_ps, msq_ps


def _ln_finish(nc, h, hb, ch, g_t, gb_t, stats, ln_env):
    """Phase-2 LayerNorm: rstd chain, broadcast, apply (+optional bf16 cast).
    In-place over the feature (partition) axis of h[:, :, chunk]."""
    ones_row128, ones2, inv_d_col, tmp, smalls, ps8 = ln_env
    chs = slice(ch * CH, (ch + 1) * CH)
    mean_ps, msq_ps = stats
    mean_sb = smalls.tile([1, CH], FP32, tag="lnrow", bufs=5, name="mean_sb")
    nc.vector.tensor_copy(mean_sb, mean_ps[0:1, :])
    sqm = smalls.tile([1, CH], FP32, tag="lnrow", bufs=5, name="sqm")
    nc.vector.tensor_tensor(out=sqm, in0=mean_sb, in1=mean_sb, op=OP.mult)
    var = smalls.tile([1, CH], FP32, tag="lnrow", bufs=5, name="var")
    nc.vector.scalar_tensor_tensor(out=var, in0=msq_ps[0:1, :],
                                   scalar=EPS, in1=sqm, op0=OP.add,
                                   op1=OP.subtract)
    mr = smalls.tile([1, CH], FP32R, tag="lnrow", bufs=5, name="mr")
    with nc.allow_low_precision(reason="DVE pow for rstd"):
        nc.vector.tensor_scalar(out=mr, in0=var, scalar1=-0.5, scalar2=None,
                                op0=OP.pow)  # rstd = var**-0.5
    # rhs2: row 0 = mean*rstd, row 1 = ones (for the fused K=2 c2 matmul)
    rhs2 = smalls.tile([2, CH], FP32R, tag="rhs2", name="rhs2")
    nc.vector.tensor_copy(rhs2, ones2)
    nc.vector.tensor_tensor(out=rhs2[0:1, :], in0=mean_sb, in1=mr,
                            op=OP.mult)
    rstd_b = ps8.tile([P, CH], FP32, tag="ps", name="rstd_b")
    nc.tensor.matmul(rstd_b, lhsT=ones_row128, rhs=mr, start=True, stop=True)
    for mt in range(DT):
        c2 = ps8.tile([P, CH], FP32, tag="ps", name="c2")
        nc.tensor.matmul(c2, lhsT=gb_t[0:2, mt * P:(mt + 1) * P],
                         rhs=rhs2[0:2, :], start=True, stop=True)
        t2 = tmp.tile([P, CH], FP32, tag="t2")
        nc.vector.tensor_tensor(out=t2, in0=h[:, mt, chs], in1=rstd_b,
                                op=OP.mult)
        nc.vector.scalar_tensor_tensor(
            out=h[:, mt, chs], in0=t2, scalar=g_t[:, mt:mt + 1],
            in1=c2, op0=OP.mult, op1=OP.add)
        if hb is not None:
            nc.gpsimd.tensor_copy(hb[:, mt, chs], h[:, mt, chs])


# ---------------- host side ----------------

def _bf16(a):
    import ml_dtypes
    return np.ascontiguousarray(a).astype(ml_dtypes.bfloat16)


def _fp8(a):
    import ml_dtypes
    return np.ascontiguousarray(a).astype(mybir.dt.np(FP8))


def _pos_encoding_np():
    pos = np.arange(S, dtype=np.float32)[:, None]
    i = np.arange(D // 2, dtype=np.float32)[None, :]
    denom_s = np.power(np.float32(10000.0), (2.0 * i / D).astype(np.float32))
    denom_c = np.power(np.float32(10000.0),
                       (2.0 * (i + 1.0) / D).astype(np.float32))
    pe = np.zeros((S, D), np.float32)
    pe[:, 0::2] = np.sin(pos / denom_s)
    pe[:, 1::2] = np.cos(pos / denom_c)
    return pe  # [S, D]


def _prep_shared(emb, Wq, bq, Wk, bk, Wv, bv, Wo, bo, W1, b1, W2, b2,
                 g1, be1, g2, be2):
    f32 = np.float32
    scale = f32(1.0 / np.sqrt(DN))

    def cols(a, nt):  # [L, nt*128] -> [L, 128, nt]
        return np.ascontiguousarray(
            np.asarray(a).reshape(L, nt, P).transpose(0, 2, 1)).astype(f32)

    def blocks(a):  # [L, D(k), D(m)] -> [L, P(k), DT(mt), DT(kt), P(m)]
        return _bf16(
            a.reshape(L, DT, P, DT, P).transpose(0, 2, 3, 1, 4))

    Wq, Wk, Wv, Wo = (np.asarray(a)[:L] for a in (Wq, Wk, Wv, Wo))
    W1, W2 = np.asarray(W1)[:L], np.asarray(W2)[:L]
    bq, bk, bv, bo = (np.asarray(a)[:L] for a in (bq, bk, bv, bo))
    b1, b2 = np.asarray(b1)[:L], np.asarray(b2)[:L]
    g1, be1, g2, be2 = (np.asarray(a)[:L] for a in (g1, be1, g2, be2))

    wq_h = blocks(Wq.transpose(0, 2, 1, 3).reshape(L, D, D) * scale)
    wk_h = blocks(Wk.transpose(0, 2, 1, 3).reshape(L, D, D))
    wo_h = blocks(Wo.astype(f32))
    wv_h = _bf16(Wv.transpose(0, 2, 1, 3).reshape(L, DT, P, D)
                 .transpose(0, 2, 1, 3))
    w1_h = _bf16(W1.reshape(L, DT, P, FT, P).transpose(0, 3, 2, 1, 4))
    w2_h = _bf16(W2.reshape(L, FT, P, DT, P)
                 .transpose(0, 3, 2, 1, 4))

    return dict(
        wq=wq_h, wk=wk_h, wv=wv_h, wo=wo_h, w1=w1_h, w2=w2_h,
        bq_c=cols(bq.reshape(L, D) * scale, DT),
        bk_c=cols(bk.reshape(L, D), DT),
        bv_r=np.ascontiguousarray(bv.reshape(L, D)).astype(f32),
        bo_c=cols(bo, DT),
        b1_c=cols(b1, FT),
        b2_c=cols(b2, DT),
        g1_c=cols(g1, DT),
        g2_c=cols(g2, DT),
        gb1=np.ascontiguousarray(np.stack([-g1, be1], axis=1)).astype(f32),
        gb2=np.ascontiguousarray(np.stack([-g2, be2], axis=1)).astype(f32),
    )


def _prep_percore(x, padding_mask, emb):
    """Per-core h0 (= emb[x] + pos-encoding, feature-major) and mask."""
    emb = np.asarray(emb, dtype=np.float32)
    x_i = np.asarray(x).astype(np.int64)
    mask_f = (1.0 - np.asarray(padding_mask).astype(np.float32))
    pe = _pos_encoding_np()                       # [S, D]
    per = []
    for c in range(NCORES):
        xs = x_i[c * BL:(c + 1) * BL]             # [BL, S]
        hc = emb[xs.reshape(-1)] + np.tile(pe, (BL, 1))   # [T, D]
        h0 = np.ascontiguousarray(
            hc.T.reshape(DT, P, T).transpose(1, 0, 2)).astype(np.float32)
        ms = mask_f[c * BL:(c + 1) * BL]          # [BL, S, S]
        # mmask[p, b, tci, s] = (1-mask)[b, s, tci*128+p]
        mt = _fp8(
            ms.transpose(0, 2, 1).reshape(BL, SC, P, S).transpose(2, 0, 1, 3))
        per.append(dict(h0=h0, mmask=mt))
    return per


def kernel(x, padding_mask, emb, Wq, bq, Wk, bk, Wv, bv, Wo, bo,
           W1, b1, W2, b2, g1, be1, g2, be2):
    if "nc" not in _PROGRAM_CACHE:
        _PROGRAM_CACHE["nc"] = _build_program(reps=1)
    nc = _PROGRAM_CACHE["nc"]

    shared = _prep_shared(emb, Wq, bq, Wk, bk, Wv, bv, Wo, bo, W1, b1, W2, b2,
                          g1, be1, g2, be2)
    per = _prep_percore(x, padding_mask, emb)

    in_maps = []
    for c in range(NCORES):
        m = dict(shared)
        m.update(per[c])
        in_maps.append(m)

    res = run_bass_kernel_spmd(nc, in_maps, core_ids=list(range(NCORES)))

    outs = []
    for c in range(NCORES):
        oc = res.results[c]["out"]                    # [P, DT, T]
        hc = oc.transpose(2, 1, 0).reshape(T, D)      # [T, D]
        outs.append(hc.reshape(BL, S, D))
    return np.concatenate(outs, axis=0).astype(np.float32)


if __name__ == "__main__":
    pass


# revision 18
# speedup vs baseline: 30.7781x; 1.0209x over previous
"""Trainium2 Bass kernel for nn_Encoder: 6-layer post-LN transformer encoder.

Sharding: pure data-parallel over batch across 8 NeuronCores (2 sequences per
core), zero collectives. On-device layout is feature-major ([D on partitions,
tokens on free dim]) so every projection uses the stored weight directly as the
matmul stationary operand and per-feature biases are per-partition scalars.

The embedding gather + positional encoding happen on the host (3 MB of
activations per core instead of a 98 MB table). All matmul operands (weights
and activations) are bf16 with fp32 PSUM accumulation; the residual stream and
LayerNorm stay fp32. Weight tensors are laid out so each layer's projection
weights arrive in one 9 KB-per-partition-line DMA.

The two sequences a core owns are independent streams (token chunks ch=0/1);
per-stage work is emitted chunk-interleaved so one stream's matmuls hide the
other stream's softmax/LayerNorm latency chains.

Attention computes transposed scores [t, s] per head as two concurrent K=64
row-tiled matmuls (partition halves of q/k), exponentiates without
max-subtraction (scores are O(1) by construction; masking is exp(s)*(1-m),
exact since exp(-1e9) underflows to 0), and contracts PV with M=65 matmuls
whose 65th row is the softmax denominator (ones-column of V). The pair loop
is software-pipelined with a skew of 2 so score matmuls of later pairs fill
the PE while earlier pairs' exp/mask/normalize chains complete. The FFN keeps
PSUM pressure at ~2 banks by materializing relu outputs in SBUF and running
the W2 contraction output-stationary (one accumulator bank per output tile),
which lets attention, LayerNorm, and FFN stages of the two chunks overlap.

LayerNorm reduces over the feature (partition) axis with ones-matmuls
(E[x^2]-E[x]^2+eps), then applies (z*rstd)*g + (-g*mean*rstd + b) where the
per-token row factors are broadcast across partitions via K=1/K=2 PE outer
products and per-feature factors are per-partition scalars.
"""

import os
import sys

import numpy as np

for _p in ("/root/.axon_site/_ro/trn_rl_repo", "/opt/trn_rl_repo"):
    if os.path.isdir(_p) and _p not in sys.path:
        sys.path.append(_p)

import concourse.bass as bass  # noqa: E402
import concourse.mybir as mybir  # noqa: E402
import concourse.tile as tile  # noqa: E402
from concourse import bacc  # noqa: E402
from concourse.bass_utils import run_bass_kernel_spmd  # noqa: E402

# Problem constants (hardcoded per harness contract).
V, D, H, F = 32000, 768, 12, 3072
L = int(os.environ.get("ENC_LAYERS", "6"))
DN = D // H            # 64
B, S = 16, 512
NCORES = 8
BL = B // NCORES       # 2 sequences per core
T = BL * S             # 1024 tokens per core
P = 128
DT = D // P            # 6 feature tiles
TC = T // P            # 8 token chunks
SC = S // P            # 4 chunks per sequence
FT = F // P            # 24 ff tiles
NCH = 2                # T split into chunks of 512 (= one sequence each)
CH = T // NCH          # 512
EPS = 1e-5
FP32 = mybir.dt.float32
FP32R = mybir.dt.float32r
BF16 = mybir.dt.bfloat16
FP8 = mybir.dt.float8e4
I32 = mybir.dt.int32

AF = mybir.ActivationFunctionType
OP = mybir.AluOpType

_PROGRAM_CACHE = {}


def _build_program(reps=None):
    """Build + compile the per-core program. reps>1 reruns the whole encoder
    (h reloaded from DRAM each rep) for marginal-time measurement."""
    if reps is None:
        reps = int(os.environ.get("ENC_REPS", "1"))
    nc = bacc.Bacc("TRN2", target_bir_lowering=False, debug=False,
                   num_devices=NCORES)

    io = {}

    def inp(name, shape, dtype=FP32):
        io[name] = nc.declare_dram_parameter(name, list(shape), dtype,
                                             isOutput=False)

    inp("h0", [P, DT, T], FP32R)          # host: (emb[x]+pe), feature-major
    inp("mmask", [P, BL, SC, S], FP8)     # host: 1 - padding_mask, transposed
    inp("wq", [L, P, DT, DT, P], BF16)    # [l, 128k, mtile, ktile, 128m]
    inp("wk", [L, P, DT, DT, P], BF16)
    inp("wo", [L, P, DT, DT, P], BF16)
    inp("wv", [L, P, DT, D], BF16)        # [l, 128k, ktile, 768m]
    inp("w1", [L, FT, P, DT, P], BF16)    # [l, mtile, 128k, ktile, 128m]
    inp("w2", [L, DT, P, FT, P], BF16)    # [l, mtile, 128k, ktile, 128m]
    inp("bq_c", [L, P, DT])
    inp("bk_c", [L, P, DT])
    inp("bv_r", [L, D])
    inp("bo_c", [L, P, DT])
    inp("b1_c", [L, P, FT])
    inp("b2_c", [L, P, DT])
    inp("g1_c", [L, P, DT])
    inp("g2_c", [L, P, DT])
    inp("gb1", [L, 2, D], FP32R)
    inp("gb2", [L, 2, D], FP32R)
    io["out"] = nc.declare_dram_parameter("out", [P, DT, T], FP32,
                                          isOutput=True)

    with tile.TileContext(nc) as tc:
        _emit(nc, tc, io, reps)
    nc.compile()
    return nc


def _emit(nc, tc, io, reps):
    from contextlib import ExitStack

    with ExitStack() as ctx:
        singles = ctx.enter_context(tc.tile_pool(name="singles", bufs=1))
        acts = ctx.enter_context(tc.tile_pool(name="acts", bufs=1))
        wproj = ctx.enter_context(tc.tile_pool(name="wproj", bufs=1))
        wpool = ctx.enter_context(tc.tile_pool(name="wpool", bufs=4))
        w1pool = ctx.enter_context(tc.tile_pool(name="w1pool", bufs=3))
        w2pool = ctx.enter_context(tc.tile_pool(name="w2pool", bufs=2))
        ffp = ctx.enter_context(tc.tile_pool(name="ffp", bufs=1))
        lw = ctx.enter_context(tc.tile_pool(name="lw", bufs=2))
        tmp = ctx.enter_context(tc.tile_pool(name="tmp", bufs=2))
        smalls = ctx.enter_context(tc.tile_pool(name="smalls", bufs=2))
        ps8 = ctx.enter_context(tc.tile_pool(name="ps8", bufs=8,
                                             space="PSUM"))

        # ---- persistent activations (feature-major unless noted) ----
        h = acts.tile([P, DT, T], FP32R)
        hb = acts.tile([P, DT, T], BF16)   # bf16 copy of h (matmul rhs)
        q = acts.tile([P, DT, T], BF16)    # also holds attention output o
        k = acts.tile([P, DT, T], BF16)
        v = acts.tile([P, TC, H, DN + 1], BF16)  # token-major, +ones col
        o = q
        mm_sb = acts.tile([P, BL, SC, S], FP8)
        nc.sync.dma_start(mm_sb, io["mmask"][:])

        # ---- constants ----
        cst_f = singles.tile([P, 2], FP32)
        nc.vector.memset(cst_f[:, 0:1], 1.0)
        nc.vector.memset(cst_f[:, 1:2], 1.0 / D)
        cst_r = singles.tile([P, 2], FP32R)
        nc.vector.tensor_copy(cst_r, cst_f)
        inv_d_col = cst_r[:, 1:2]
        row2_f = singles.tile([2, CH], FP32)
        nc.vector.memset(row2_f, 1.0)
        ones2 = singles.tile([2, CH], FP32R)
        nc.vector.tensor_copy(ones2, row2_f)
        ones_row128 = ones2[0:1, 0:P]
        op64_f = singles.tile([65, 64], FP32)
        nc.vector.memset(op64_f[64:65, :], 1.0)
        ones_p64 = singles.tile([65, 64], FP32R)
        nc.vector.tensor_copy(ones_p64[64:65, :], op64_f[64:65, :])
        # ones column of v (written once; evictions only touch cols 0:DN)
        nc.vector.tensor_copy(v[:, :, :, DN],
                              cst_r[:, 0:1].to_broadcast((P, TC, H)))

        ln_env = (ones_row128, ones2, inv_d_col, tmp, smalls, ps8)

        for rep in range(reps):
          nc.sync.dma_start(h, io["h0"][:])
          for mt in range(DT):
              nc.gpsimd.tensor_copy(hb[:, mt, :], h[:, mt, :])
          for l in range(L):
            # per-layer bias / layernorm parameter tiles
            bq_t = lw.tile([P, DT], FP32, tag="bq")
            nc.sync.dma_start(bq_t, io["bq_c"][l])
            bk_t = lw.tile([P, DT], FP32, tag="bk")
            nc.sync.dma_start(bk_t, io["bk_c"][l])
            bo_t = lw.tile([P, DT], FP32, tag="bo")
            nc.sync.dma_start(bo_t, io["bo_c"][l])
            b1_t = lw.tile([P, FT], FP32, tag="b1")
            nc.sync.dma_start(b1_t, io["b1_c"][l])
            b2_t = lw.tile([P, DT], FP32, tag="b2")
            nc.sync.dma_start(b2_t, io["b2_c"][l])
            g1_t = lw.tile([P, DT], FP32, tag="g1")
            nc.sync.dma_start(g1_t, io["g1_c"][l])
            g2_t = lw.tile([P, DT], FP32, tag="g2")
            nc.sync.dma_start(g2_t, io["g2_c"][l])
            gb1_t = lw.tile([2, D], FP32R, tag="gb1", bufs=1)
            nc.sync.dma_start(gb1_t, io["gb1"][l])
            gb2_t = lw.tile([2, D], FP32R, tag="gb2", bufs=1)
            nc.sync.dma_start(gb2_t, io["gb2"][l])
            bv_t = lw.tile([P, D], FP32, tag="bv", bufs=1)
            bvl = io["bv_r"][l]
            nc.sync.dma_start(
                bv_t, bass.AP(tensor=bvl.tensor, offset=bvl.offset,
                              ap=[[0, P]] + list(bvl.ap)))

            # per-layer wo/wv weights: one big DMA each (>=1.5KB lines)
            wo_t = wproj.tile([P, DT, DT, P], BF16, tag="wo")
            nc.sync.dma_start(wo_t, io["wo"][l])
            wv_t = wproj.tile([P, DT, D], BF16, tag="wv")
            nc.sync.dma_start(wv_t, io["wv"][l])

            # ---------- q/k projections (weights streamed per m-tile) ----
            for wname, bias_t, dst in (("wq", bq_t, q), ("wk", bk_t, k)):
                for mt in range(DT):
                    wt = wpool.tile([P, DT, P], BF16, tag="wblk")
                    nc.sync.dma_start(wt, io[wname][l][:, mt])
                    for ch in range(NCH):
                        ps = ps8.tile([P, CH], FP32, tag="ps")
                        for kt in range(DT):
                            nc.tensor.matmul(
                                ps, lhsT=wt[:, kt, :],
                                rhs=hb[:, kt, ch * CH:(ch + 1) * CH],
                                start=(kt == 0), stop=(kt == DT - 1))
                        nc.vector.tensor_scalar(
                            out=dst[:, mt, ch * CH:(ch + 1) * CH], in0=ps,
                            scalar1=bias_t[:, mt:mt + 1], scalar2=None,
                            op0=OP.add)
            # v (token-major): out[t_chunk, features], ktile weights
            HD = D // 2
            for tch in range(TC):
                for nh in range(2):
                    ps = ps8.tile([P, CH], FP32, tag="ps")
                    psn = ps[:, :HD]
                    for kt in range(DT):
                        nc.tensor.matmul(
                            psn, lhsT=hb[:, kt, tch * P:(tch + 1) * P],
                            rhs=wv_t[:, kt, nh * HD:(nh + 1) * HD],
                            start=(kt == 0), stop=(kt == DT - 1))
                    nc.vector.tensor_tensor(
                        out=v[:, tch, nh * (H // 2):(nh + 1) * (H // 2),
                              0:DN],
                        in0=psn.rearrange("p (hh e) -> p hh e", e=DN),
                        in1=bv_t[:, nh * HD:(nh + 1) * HD].rearrange(
                            "p (hh e) -> p hh e", e=DN),
                        op=OP.add)

            # ---------- attention ----------
            # software-pipelined over head pairs: scores/exp/mask of pair
            # i+1 are emitted before PV/normalize of pair i so the PE has
            # score matmuls to run while pair i's softmax chain completes.
            with tc.tile_pool(name="attp", bufs=6) as attp:
                pend = []

                def produce(bb, hp):
                    exs = [attp.tile([P, SC, S], BF16, tag="ex",
                                     name=f"ex{i}") for i in range(2)]
                    for tci in range(SC):
                        sts = []
                        for hh in range(2):
                            pr = slice(hh * 64, (hh + 1) * 64)
                            st = ps8.tile([P, S], FP32, tag="ps",
                                          name=f"st{hh}")
                            nc.tensor.matmul(
                                st,
                                lhsT=k[pr, hp, bb * S + tci * P:
                                       bb * S + (tci + 1) * P],
                                rhs=q[pr, hp, bb * S:(bb + 1) * S],
                                start=True, stop=True,
                                tile_position=(hh * 64, 0))
                            sts.append(st)
                        for hh in range(2):
                            nc.scalar.activation(exs[hh][:, tci, :],
                                                 sts[hh], AF.Exp)
                            eng = nc.vector if hh == 0 else nc.gpsimd
                            eng.tensor_tensor(
                                out=exs[hh][:, tci, :],
                                in0=exs[hh][:, tci, :],
                                in1=mm_sb[:, bb, tci, :], op=OP.mult)
                    return exs

                def consume(bb, hp, exs):
                    pvs = [ps8.tile([65, S], FP32, tag="ps",
                                       name=f"pv{i}") for i in range(2)]
                    for tci in range(SC):
                        tg = bb * SC + tci
                        for hh in range(2):
                            hd = 2 * hp + hh
                            nc.tensor.matmul(
                                pvs[hh],
                                lhsT=v[:, tg, hd, :],
                                rhs=exs[hh][:, tci, :],
                                start=(tci == 0), stop=(tci == SC - 1))
                    rcs = [smalls.tile([65, S], FP32R, tag="rc",
                                       name=f"rc{i}") for i in range(2)]
                    with nc.allow_low_precision(
                            reason="fp32r softmax denominators"):
                        nc.vector.reciprocal(rcs[0][64:65, :],
                                             pvs[0][64:65, :])
                        nc.vector.reciprocal(rcs[1][64:65, :],
                                             pvs[1][64:65, :])
                    bcs = [ps8.tile([64, S], FP32, tag="ps",
                                       name=f"bc{i}") for i in range(2)]
                    nc.tensor.matmul(bcs[0], lhsT=ones_p64[64:65, :],
                                     rhs=rcs[0][64:65, :], start=True,
                                     stop=True)
                    nc.tensor.matmul(bcs[1], lhsT=ones_p64[64:65, :],
                                     rhs=rcs[1][64:65, :], start=True,
                                     stop=True)
                    # head even: normalize straight into o[0:64]
                    nc.vector.tensor_copy(
                        o[0:64, hp, bb * S:(bb + 1) * S], pvs[0][0:64, :])
                    nc.vector.tensor_tensor(
                        out=o[0:64, hp, bb * S:(bb + 1) * S],
                        in0=o[0:64, hp, bb * S:(bb + 1) * S],
                        in1=bcs[0], op=OP.mult)
                    # head odd: normalize at partitions 0-63, then
                    # DMA-shift into partitions 64-127 of o
                    ot = tmp.tile([64, S], BF16, tag="ot")
                    nc.scalar.activation(ot, pvs[1][0:64, :], AF.Copy)
                    nc.vector.tensor_tensor(out=ot, in0=ot, in1=bcs[1],
                                            op=OP.mult)
                    nc.sync.dma_start(
                        o[64:128, hp, bb * S:(bb + 1) * S], ot)

                for bb in range(BL):
                    for hp in range(DT):  # head pair: heads 2hp, 2hp+1
                        exs = produce(bb, hp)
                        if len(pend) >= 2:
                            consume(*pend.pop(0))
                        pend.append((bb, hp, exs))
                while pend:
                    consume(*pend.pop(0))

            # ---------- Wo + residual + LN1, FFN + residual + LN2 ----
            # Each LayerNorm is split into stats (PE reduction) and finish
            # (scalar chain + apply); finishes are emitted after the other
            # chunk's matmul stage so the chains hide under PE work.
            st1 = []
            for ch in range(NCH):
                for mt in range(DT):
                    ps = ps8.tile([P, CH], FP32, tag="ps")
                    for kt in range(DT):
                        nc.tensor.matmul(
                            ps, lhsT=wo_t[:, mt, kt, :],
                            rhs=o[:, kt, ch * CH:(ch + 1) * CH],
                            start=(kt == 0), stop=(kt == DT - 1))
                    nc.vector.scalar_tensor_tensor(
                        out=h[:, mt, ch * CH:(ch + 1) * CH], in0=ps,
                        scalar=bo_t[:, mt:mt + 1],
                        in1=h[:, mt, ch * CH:(ch + 1) * CH],
                        op0=OP.add, op1=OP.add)
                st1.append(_ln_stats(nc, h, ch, ln_env))
            for ch in range(NCH):
                _ln_finish(nc, h, hb, ch, g1_t, gb1_t, st1[ch], ln_env)

            st2 = []
            for ch in range(NCH):
                # phase 1: ff = relu((h W1 + b1) / FSC), stored fp8 in SBUF
                ff_all = ffp.tile([P, FT, CH], BF16, tag="ffall")
                for m in range(FT):
                    w1t = w1pool.tile([P, DT, P], BF16, tag="w1")
                    nc.sync.dma_start(w1t, io["w1"][l, m])
                    ps = ps8.tile([P, CH], FP32, tag="ps")
                    for kt in range(DT):
                        nc.tensor.matmul(
                            ps, lhsT=w1t[:, kt, :],
                            rhs=hb[:, kt, ch * CH:(ch + 1) * CH],
                            start=(kt == 0), stop=(kt == DT - 1))
                    nc.scalar.activation(ff_all[:, m, :], ps, AF.Relu,
                                         bias=b1_t[:, m:m + 1])
                # phase 2: W2-major accumulation, one PSUM bank per out tile
                for mt in range(DT):
                    w2m = w2pool.tile([P, FT, P], BF16, tag="w2")
                    nc.sync.dma_start(w2m, io["w2"][l, mt])
                    acc = ps8.tile([P, CH], FP32, tag="ps", name="acc")
                    for m in range(FT):
                        nc.tensor.matmul(
                            acc, lhsT=w2m[:, m, :],
                            rhs=ff_all[:, m, :],
                            start=(m == 0), stop=(m == FT - 1))
                    nc.vector.scalar_tensor_tensor(
                        out=h[:, mt, ch * CH:(ch + 1) * CH], in0=acc,
                        scalar=b2_t[:, mt:mt + 1],
                        in1=h[:, mt, ch * CH:(ch + 1) * CH],
                        op0=OP.add, op1=OP.add)
                st2.append(_ln_stats(nc, h, ch, ln_env))
            for ch in range(NCH):
                _ln_finish(nc, h, hb if l < L - 1 else None, ch, g2_t,
                           gb2_t, st2[ch], ln_env)

        nc.sync.dma_start(io["out"][:], h[:].bitcast(FP32))


def _ln_stats(nc, h, ch, ln_env):
    """Phase-1 LayerNorm: per-token mean and mean-square via PE reduction."""
    ones_row128, ones2, inv_d_col, tmp, smalls, ps8 = ln_env
    chs = slice(ch * CH, (ch + 1) * CH)
    mean_ps = ps8.tile([1, CH], FP32, tag="ps", name="mean_ps")
    msq_ps = ps8.tile([1, CH], FP32, tag="ps", name="msq_ps")
    for mt in range(DT):
        sq = tmp.tile([P, CH], FP32R, tag="sq")
        nc.scalar.activation(sq, h[:, mt, chs], AF.Square)
        nc.tensor.matmul(mean_ps[0:1, :], lhsT=inv_d_col,
                         rhs=h[:, mt, chs], start=(mt == 0),
                         stop=(mt == DT - 1))
        nc.tensor.matmul(msq_ps[0:1, :], lhsT=inv_d_col, rhs=sq,
                         start=(mt == 0), stop=(mt == DT - 1))
    return mean_ps, msq_ps


def _ln_finish(nc, h, hb, ch, g_t, gb_t, stats, ln_env):
    """Phase-2 LayerNorm: rstd chain, broadcast, apply (+optional bf16 cast).
    In-place over the feature (partition) axis of h[:, :, chunk]."""
    ones_row128, ones2, inv_d_col, tmp, smalls, ps8 = ln_env
    chs = slice(ch * CH, (ch + 1) * CH)
    mean_ps, msq_ps = stats
    mean_sb = smalls.tile([1, CH], FP32, tag="lnrow", bufs=4, name="mean_sb")
    nc.vector.tensor_copy(mean_sb, mean_ps[0:1, :])
    sqm = smalls.tile([1, CH], FP32, tag="lnrow", bufs=4, name="sqm")
    nc.vector.tensor_tensor(out=sqm, in0=mean_sb, in1=mean_sb, op=OP.mult)
    var = smalls.tile([1, CH], FP32, tag="lnrow", bufs=4, name="var")
    nc.vector.scalar_tensor_tensor(out=var, in0=msq_ps[0:1, :],
                                   scalar=EPS, in1=sqm, op0=OP.add,
                                   op1=OP.subtract)
    lnv = smalls.tile([1, CH], FP32, tag="lnrow", bufs=4, name="lnv")
    nc.scalar.activation(lnv, var, AF.Ln)
    mr = smalls.tile([1, CH], FP32R, tag="lnrow", bufs=4, name="mr")
    nc.scalar.activation(mr, lnv, AF.Exp, scale=-0.5)  # rstd
    # rhs2: row 0 = mean*rstd, row 1 = ones (for the fused K=2 c2 matmul)
    rhs2 = smalls.tile([2, CH], FP32R, tag="rhs2", name="rhs2")
    nc.vector.tensor_copy(rhs2, ones2)
    nc.vector.tensor_tensor(out=rhs2[0:1, :], in0=mean_sb, in1=mr,
                            op=OP.mult)
    rstd_b = ps8.tile([P, CH], FP32, tag="ps", name="rstd_b")
    nc.tensor.matmul(rstd_b, lhsT=ones_row128, rhs=mr, start=True, stop=True)
    for mt in range(DT):
        c2 = ps8.tile([P, CH], FP32, tag="ps", name="c2")
        nc.tensor.matmul(c2, lhsT=gb_t[0:2, mt * P:(mt + 1) * P],
                         rhs=rhs2[0:2, :], start=True, stop=True)
        t2 = tmp.tile([P, CH], FP32, tag="t2")
        nc.vector.tensor_tensor(out=t2, in0=h[:, mt, chs], in1=rstd_b,
                                op=OP.mult)
        nc.vector.scalar_tensor_tensor(
            out=h[:, mt, chs], in0=t2, scalar=g_t[:, mt:mt + 1],
            in1=c2, op0=OP.mult, op1=OP.add)
        if hb is not None:
            nc.gpsimd.tensor_copy(hb[:, mt, chs], h[:, mt, chs])


# ---------------- host side ----------------

def _bf16(a):
    import ml_dtypes
    return np.ascontiguousarray(a).astype(ml_dtypes.bfloat16)


def _fp8(a):
    import ml_dtypes
    return np.ascontiguousarray(a).astype(mybir.dt.np(FP8))


def _pos_encoding_np():
    pos = np.arange(S, dtype=np.float32)[:, None]
    i = np.arange(D // 2, dtype=np.float32)[None, :]
    denom_s = np.power(np.float32(10000.0), (2.0 * i / D).astype(np.float32))
    denom_c = np.power(np.float32(10000.0),
                       (2.0 * (i + 1.0) / D).astype(np.float32))
    pe = np.zeros((S, D), np.float32)
    pe[:, 0::2] = np.sin(pos / denom_s)
    pe[:, 1::2] = np.cos(pos / denom_c)
    return pe  # [S, D]


def _prep_shared(emb, Wq, bq, Wk, bk, Wv, bv, Wo, bo, W1, b1, W2, b2,
                 g1, be1, g2, be2):
    f32 = np.float32
    scale = f32(1.0 / np.sqrt(DN))

    def cols(a, nt):  # [L, nt*128] -> [L, 128, nt]
        return np.ascontiguousarray(
            np.asarray(a).reshape(L, nt, P).transpose(0, 2, 1)).astype(f32)

    def blocks(a):  # [L, D(k), D(m)] -> [L, P(k), DT(mt), DT(kt), P(m)]
        return _bf16(
            a.reshape(L, DT, P, DT, P).transpose(0, 2, 3, 1, 4))

    Wq, Wk, Wv, Wo = (np.asarray(a)[:L] for a in (Wq, Wk, Wv, Wo))
    W1, W2 = np.asarray(W1)[:L], np.asarray(W2)[:L]
    bq, bk, bv, bo = (np.asarray(a)[:L] for a in (bq, bk, bv, bo))
    b1, b2 = np.asarray(b1)[:L], np.asarray(b2)[:L]
    g1, be1, g2, be2 = (np.asarray(a)[:L] for a in (g1, be1, g2, be2))

    wq_h = blocks(Wq.transpose(0, 2, 1, 3).reshape(L, D, D) * scale)
    wk_h = blocks(Wk.transpose(0, 2, 1, 3).reshape(L, D, D))
    wo_h = blocks(Wo.astype(f32))
    wv_h = _bf16(Wv.transpose(0, 2, 1, 3).reshape(L, DT, P, D)
                 .transpose(0, 2, 1, 3))
    w1_h = _bf16(W1.reshape(L, DT, P, FT, P).transpose(0, 3, 2, 1, 4))
    w2_h = _bf16(W2.reshape(L, FT, P, DT, P)
                 .transpose(0, 3, 2, 1, 4))

    return dict(
        wq=wq_h, wk=wk_h, wv=wv_h, wo=wo_h, w1=w1_h, w2=w2_h,
        bq_c=cols(bq.reshape(L, D) * scale, DT),
        bk_c=cols(bk.reshape(L, D), DT),
        bv_r=np.ascontiguousarray(bv.reshape(L, D)).astype(f32),
        bo_c=cols(bo, DT),
        b1_c=cols(b1, FT),
        b2_c=cols(b2, DT),
        g1_c=cols(g1, DT),
        g2_c=cols(g2, DT),
        gb1=np.ascontiguousarray(np.stack([-g1, be1], axis=1)).astype(f32),
        gb2=np.ascontiguousarray(np.stack([-g2, be2], axis=1)).astype(f32),
    )


def _prep_percore(x, padding_mask, emb):
    """Per-core h0 (= emb[x] + pos-encoding, feature-major) and mask."""
    emb = np.asarray(emb, dtype=np.float32)
    x_i = np.asarray(x).astype(np.int64)
    mask_f = (1.0 - np.asarray(padding_mask).astype(np.float32))
    pe = _pos_encoding_np()                       # [S, D]
    per = []
    for c in range(NCORES):
        xs = x_i[c * BL:(c + 1) * BL]             # [BL, S]
        hc = emb[xs.reshape(-1)] + np.tile(pe, (BL, 1))   # [T, D]
        h0 = np.ascontiguousarray(
            hc.T.reshape(DT, P, T).transpose(1, 0, 2)).astype(np.float32)
        ms = mask_f[c * BL:(c + 1) * BL]          # [BL, S, S]
        # mmask[p, b, tci, s] = (1-mask)[b, s, tci*128+p]
        mt = _fp8(
            ms.transpose(0, 2, 1).reshape(BL, SC, P, S).transpose(2, 0, 1, 3))
        per.append(dict(h0=h0, mmask=mt))
    return per


def kernel(x, padding_mask, emb, Wq, bq, Wk, bk, Wv, bv, Wo, bo,
           W1, b1, W2, b2, g1, be1, g2, be2):
    if "nc" not in _PROGRAM_CACHE:
        _PROGRAM_CACHE["nc"] = _build_program(reps=1)
    nc = _PROGRAM_CACHE["nc"]

    shared = _prep_shared(emb, Wq, bq, Wk, bk, Wv, bv, Wo, bo, W1, b1, W2, b2,
                          g1, be1, g2, be2)
    per = _prep_percore(x, padding_mask, emb)

    in_maps = []
    for c in range(NCORES):
        m = dict(shared)
        m.update(per[c])
        in_maps.append(m)

    res = run_bass_kernel_spmd(nc, in_maps, core_ids=list(range(NCORES)))

    outs = []
    for c in range(NCORES):
        oc = res.results[c]["out"]                    # [P, DT, T]
        hc = oc.transpose(2, 1, 0).reshape(T, D)      # [T, D]
        outs.append(hc.reshape(BL, S, D))
    return np.concatenate(outs, axis=0).astype(np.float32)


if __name__ == "__main__":
    pass


# revision 20
# speedup vs baseline: 31.3427x; 1.0183x over previous
"""Trainium2 Bass kernel for nn_Encoder: 6-layer post-LN transformer encoder.

Sharding: pure data-parallel over batch across 8 NeuronCores (2 sequences per
core), zero collectives. On-device layout is feature-major ([D on partitions,
tokens on free dim]) so every projection uses the stored weight directly as the
matmul stationary operand and per-feature biases are per-partition scalars.

The embedding gather + positional encoding happen on the host (3 MB of
activations per core instead of a 98 MB table). All matmul operands (weights
and activations) are bf16 with fp32 PSUM accumulation; the residual stream and
LayerNorm stay fp32. Weight tensors are laid out so each layer's projection
weights arrive in one 9 KB-per-partition-line DMA.

The two sequences a core owns are independent streams (token chunks ch=0/1);
per-stage work is emitted chunk-interleaved so one stream's matmuls hide the
other stream's softmax/LayerNorm latency chains.

Attention computes transposed scores [t, s] per head as two concurrent K=64
row-tiled matmuls (partition halves of q/k), exponentiates without
max-subtraction (scores are O(1) by construction; masking is exp(s)*(1-m),
exact since exp(-1e9) underflows to 0), and contracts PV with M=65 matmuls
whose 65th row is the softmax denominator (ones-column of V). The pair loop
is software-pipelined with a skew of 2 so score matmuls of later pairs fill
the PE while earlier pairs' exp/mask/normalize chains complete. The FFN keeps
PSUM pressure at ~2 banks by materializing relu outputs in SBUF and running
the W2 contraction output-stationary (one accumulator bank per output tile),
which lets attention, LayerNorm, and FFN stages of the two chunks overlap.

LayerNorm reduces over the feature (partition) axis with ones-matmuls
(E[x^2]-E[x]^2+eps), then applies (z*rstd)*g + (-g*mean*rstd + b) where the
per-token row factors are broadcast across partitions via K=1/K=2 PE outer
products and per-feature factors are per-partition scalars.
"""

import os
import sys

import numpy as np

for _p in ("/root/.axon_site/_ro/trn_rl_repo", "/opt/trn_rl_repo"):
    if os.path.isdir(_p) and _p not in sys.path:
        sys.path.append(_p)

import concourse.bass as bass  # noqa: E402
import concourse.mybir as mybir  # noqa: E402
import concourse.tile as tile  # noqa: E402
from concourse import bacc  # noqa: E402
from concourse.bass_utils import run_bass_kernel_spmd  # noqa: E402

# Problem constants (hardcoded per harness contract).
V, D, H, F = 32000, 768, 12, 3072
L = int(os.environ.get("ENC_LAYERS", "6"))
DN = D // H            # 64
B, S = 16, 512
NCORES = 8
BL = B // NCORES       # 2 sequences per core
T = BL * S             # 1024 tokens per core
P = 128
DT = D // P            # 6 feature tiles
TC = T // P            # 8 token chunks
SC = S // P            # 4 chunks per sequence
FT = F // P            # 24 ff tiles
NCH = 2                # T split into chunks of 512 (= one sequence each)
CH = T // NCH          # 512
EPS = 1e-5
FP32 = mybir.dt.float32
FP32R = mybir.dt.float32r
BF16 = mybir.dt.bfloat16
FP8 = mybir.dt.float8e4
I32 = mybir.dt.int32

AF = mybir.ActivationFunctionType
OP = mybir.AluOpType

_PROGRAM_CACHE = {}


def _build_program(reps=None):
    """Build + compile the per-core program. reps>1 reruns the whole encoder
    (h reloaded from DRAM each rep) for marginal-time measurement."""
    if reps is None:
        reps = int(os.environ.get("ENC_REPS", "1"))
    nc = bacc.Bacc("TRN2", target_bir_lowering=False, debug=False,
                   num_devices=NCORES)

    io = {}

    def inp(name, shape, dtype=FP32):
        io[name] = nc.declare_dram_parameter(name, list(shape), dtype,
                                             isOutput=False)

    inp("h0", [P, DT, T], FP32R)          # host: (emb[x]+pe), feature-major
    inp("mmask", [P, BL, SC, S], FP8)     # host: 1 - padding_mask, transposed
    inp("wq", [L, P, DT, DT, P], BF16)    # [l, 128k, mtile, ktile, 128m]
    inp("wk", [L, P, DT, DT, P], BF16)
    inp("wo", [L, P, DT, DT, P], BF16)
    inp("wv", [L, P, DT, D], BF16)        # [l, 128k, ktile, 768m]
    inp("w1", [L, FT, P, DT, P], BF16)    # [l, mtile, 128k, ktile, 128m]
    inp("w2", [L, DT, P, FT, P], BF16)    # [l, mtile, 128k, ktile, 128m]
    inp("bq_c", [L, P, DT])
    inp("bk_c", [L, P, DT])
    inp("bv_r", [L, D])
    inp("bo_c", [L, P, DT])
    inp("b1_c", [L, P, FT])
    inp("b2_c", [L, P, DT])
    inp("g1_c", [L, P, DT])
    inp("g2_c", [L, P, DT])
    inp("gb1", [L, 2, D], FP32R)
    inp("gb2", [L, 2, D], FP32R)
    io["out"] = nc.declare_dram_parameter("out", [P, DT, T], FP32,
                                          isOutput=True)

    with tile.TileContext(nc) as tc:
        _emit(nc, tc, io, reps)
    nc.compile()
    return nc


def _emit(nc, tc, io, reps):
    from contextlib import ExitStack

    with ExitStack() as ctx:
        singles = ctx.enter_context(tc.tile_pool(name="singles", bufs=1))
        acts = ctx.enter_context(tc.tile_pool(name="acts", bufs=1))
        wproj = ctx.enter_context(tc.tile_pool(name="wproj", bufs=1))
        wpool = ctx.enter_context(tc.tile_pool(name="wpool", bufs=4))
        w1pool = ctx.enter_context(tc.tile_pool(name="w1pool", bufs=3))
        w2pool = ctx.enter_context(tc.tile_pool(name="w2pool", bufs=2))
        ffp = ctx.enter_context(tc.tile_pool(name="ffp", bufs=1))
        lw = ctx.enter_context(tc.tile_pool(name="lw", bufs=2))
        tmp = ctx.enter_context(tc.tile_pool(name="tmp", bufs=2))
        smalls = ctx.enter_context(tc.tile_pool(name="smalls", bufs=2))
        ps8 = ctx.enter_context(tc.tile_pool(name="ps8", bufs=8,
                                             space="PSUM"))

        # ---- persistent activations (feature-major unless noted) ----
        h = acts.tile([P, DT, T], FP32R)
        hb = acts.tile([P, DT, T], BF16)   # bf16 copy of h (matmul rhs)
        q = acts.tile([P, DT, T], BF16)    # also holds attention output o
        k = acts.tile([P, DT, T], BF16)
        v = acts.tile([P, TC, H, DN + 1], BF16)  # token-major, +ones col
        o = q
        mm_sb = acts.tile([P, BL, SC, S], FP8)
        nc.sync.dma_start(mm_sb, io["mmask"][:])

        # ---- constants ----
        cst_f = singles.tile([P, 2], FP32)
        nc.vector.memset(cst_f[:, 0:1], 1.0)
        nc.vector.memset(cst_f[:, 1:2], 1.0 / D)
        cst_r = singles.tile([P, 2], FP32R)
        nc.vector.tensor_copy(cst_r, cst_f)
        inv_d_col = cst_r[:, 1:2]
        row2_f = singles.tile([2, CH], FP32)
        nc.vector.memset(row2_f, 1.0)
        ones2 = singles.tile([2, CH], FP32R)
        nc.vector.tensor_copy(ones2, row2_f)
        ones_row128 = ones2[0:1, 0:P]
        op64_f = singles.tile([65, 64], FP32)
        nc.vector.memset(op64_f[64:65, :], 1.0)
        ones_p64 = singles.tile([65, 64], FP32R)
        nc.vector.tensor_copy(ones_p64[64:65, :], op64_f[64:65, :])
        # ones column of v (written once; evictions only touch cols 0:DN)
        nc.vector.tensor_copy(v[:, :, :, DN],
                              cst_r[:, 0:1].to_broadcast((P, TC, H)))

        ln_env = (ones_row128, ones2, inv_d_col, tmp, smalls, ps8)

        for rep in range(reps):
          nc.sync.dma_start(h, io["h0"][:])
          for mt in range(DT):
              nc.gpsimd.tensor_copy(hb[:, mt, :], h[:, mt, :])
          for l in range(L):
            # per-layer bias / layernorm parameter tiles
            bq_t = lw.tile([P, DT], FP32, tag="bq")
            nc.sync.dma_start(bq_t, io["bq_c"][l])
            bk_t = lw.tile([P, DT], FP32, tag="bk")
            nc.sync.dma_start(bk_t, io["bk_c"][l])
            bo_t = lw.tile([P, DT], FP32, tag="bo")
            nc.sync.dma_start(bo_t, io["bo_c"][l])
            b1_t = lw.tile([P, FT], FP32, tag="b1")
            nc.sync.dma_start(b1_t, io["b1_c"][l])
            b2_t = lw.tile([P, DT], FP32, tag="b2")
            nc.sync.dma_start(b2_t, io["b2_c"][l])
            g1_t = lw.tile([P, DT], FP32, tag="g1")
            nc.sync.dma_start(g1_t, io["g1_c"][l])
            g2_t = lw.tile([P, DT], FP32, tag="g2")
            nc.sync.dma_start(g2_t, io["g2_c"][l])
            gb1_t = lw.tile([2, D], FP32R, tag="gb1", bufs=1)
            nc.sync.dma_start(gb1_t, io["gb1"][l])
            gb2_t = lw.tile([2, D], FP32R, tag="gb2", bufs=1)
            nc.sync.dma_start(gb2_t, io["gb2"][l])
            bv_t = lw.tile([P, D], FP32, tag="bv", bufs=1)
            bvl = io["bv_r"][l]
            nc.sync.dma_start(
                bv_t, bass.AP(tensor=bvl.tensor, offset=bvl.offset,
                              ap=[[0, P]] + list(bvl.ap)))

            # per-layer wo/wv weights: one big DMA each (>=1.5KB lines)
            wo_t = wproj.tile([P, DT, DT, P], BF16, tag="wo")
            nc.sync.dma_start(wo_t, io["wo"][l])
            wv_t = wproj.tile([P, DT, D], BF16, tag="wv")
            nc.sync.dma_start(wv_t, io["wv"][l])

            # ---------- q/k projections (weights streamed per m-tile,
            # q/k interleaved so attention pair hp only waits for m-tile hp)
            for mt in range(DT):
                for wname, bias_t, dst in (("wq", bq_t, q), ("wk", bk_t, k)):
                    wt = wpool.tile([P, DT, P], BF16, tag="wblk")
                    nc.sync.dma_start(wt, io[wname][l][:, mt])
                    for ch in range(NCH):
                        ps = ps8.tile([P, CH], FP32, tag="ps")
                        for kt in range(DT):
                            nc.tensor.matmul(
                                ps, lhsT=wt[:, kt, :],
                                rhs=hb[:, kt, ch * CH:(ch + 1) * CH],
                                start=(kt == 0), stop=(kt == DT - 1))
                        nc.vector.tensor_scalar(
                            out=dst[:, mt, ch * CH:(ch + 1) * CH], in0=ps,
                            scalar1=bias_t[:, mt:mt + 1], scalar2=None,
                            op0=OP.add)

            def project_v(tch):
                # v (token-major): out[t_chunk, features], ktile weights
                HD = D // 2
                for nh in range(2):
                    ps = ps8.tile([P, CH], FP32, tag="ps")
                    psn = ps[:, :HD]
                    for kt in range(DT):
                        nc.tensor.matmul(
                            psn, lhsT=hb[:, kt, tch * P:(tch + 1) * P],
                            rhs=wv_t[:, kt, nh * HD:(nh + 1) * HD],
                            start=(kt == 0), stop=(kt == DT - 1))
                    nc.vector.tensor_tensor(
                        out=v[:, tch, nh * (H // 2):(nh + 1) * (H // 2),
                              0:DN],
                        in0=psn.rearrange("p (hh e) -> p hh e", e=DN),
                        in1=bv_t[:, nh * HD:(nh + 1) * HD].rearrange(
                            "p (hh e) -> p hh e", e=DN),
                        op=OP.add)

            # ---------- attention ----------
            # software-pipelined over head pairs: scores/exp/mask of pair
            # i+1 are emitted before PV/normalize of pair i so the PE has
            # score matmuls to run while pair i's softmax chain completes.
            with tc.tile_pool(name="attp", bufs=6) as attp:
                pend = []

                def produce(bb, hp):
                    exs = [attp.tile([P, SC, S], BF16, tag="ex",
                                     name=f"ex{i}") for i in range(2)]
                    for tci in range(SC):
                        sts = []
                        for hh in range(2):
                            pr = slice(hh * 64, (hh + 1) * 64)
                            st = ps8.tile([P, S], FP32, tag="ps",
                                          name=f"st{hh}")
                            nc.tensor.matmul(
                                st,
                                lhsT=k[pr, hp, bb * S + tci * P:
                                       bb * S + (tci + 1) * P],
                                rhs=q[pr, hp, bb * S:(bb + 1) * S],
                                start=True, stop=True,
                                tile_position=(hh * 64, 0))
                            sts.append(st)
                        for hh in range(2):
                            nc.scalar.activation(exs[hh][:, tci, :],
                                                 sts[hh], AF.Exp)
                            eng = nc.vector if hh == 0 else nc.gpsimd
                            eng.tensor_tensor(
                                out=exs[hh][:, tci, :],
                                in0=exs[hh][:, tci, :],
                                in1=mm_sb[:, bb, tci, :], op=OP.mult)
                    return exs

                def consume(bb, hp, exs):
                    pvs = [ps8.tile([65, S], FP32, tag="ps",
                                       name=f"pv{i}") for i in range(2)]
                    for tci in range(SC):
                        tg = bb * SC + tci
                        for hh in range(2):
                            hd = 2 * hp + hh
                            nc.tensor.matmul(
                                pvs[hh],
                                lhsT=v[:, tg, hd, :],
                                rhs=exs[hh][:, tci, :],
                                start=(tci == 0), stop=(tci == SC - 1))
                    rcs = [smalls.tile([65, S], FP32R, tag="rc",
                                       name=f"rc{i}") for i in range(2)]
                    with nc.allow_low_precision(
                            reason="fp32r softmax denominators"):
                        nc.vector.reciprocal(rcs[0][64:65, :],
                                             pvs[0][64:65, :])
                        nc.vector.reciprocal(rcs[1][64:65, :],
                                             pvs[1][64:65, :])
                    bcs = [ps8.tile([64, S], FP32, tag="ps",
                                       name=f"bc{i}") for i in range(2)]
                    nc.tensor.matmul(bcs[0], lhsT=ones_p64[64:65, :],
                                     rhs=rcs[0][64:65, :], start=True,
                                     stop=True)
                    nc.tensor.matmul(bcs[1], lhsT=ones_p64[64:65, :],
                                     rhs=rcs[1][64:65, :], start=True,
                                     stop=True)
                    # head even: normalize straight into o[0:64]
                    nc.vector.tensor_copy(
                        o[0:64, hp, bb * S:(bb + 1) * S], pvs[0][0:64, :])
                    nc.vector.tensor_tensor(
                        out=o[0:64, hp, bb * S:(bb + 1) * S],
                        in0=o[0:64, hp, bb * S:(bb + 1) * S],
                        in1=bcs[0], op=OP.mult)
                    # head odd: normalize at partitions 0-63, then
                    # DMA-shift into partitions 64-127 of o
                    ot = tmp.tile([64, S], BF16, tag="ot")
                    nc.scalar.activation(ot, pvs[1][0:64, :], AF.Copy)
                    nc.vector.tensor_tensor(out=ot, in0=ot, in1=bcs[1],
                                            op=OP.mult)
                    nc.sync.dma_start(
                        o[64:128, hp, bb * S:(bb + 1) * S], ot)

                # seed two pairs so their exp chains run while the
                # v-projection matmuls (which consume() needs) execute
                pend.append((0, 0, produce(0, 0)))
                pend.append((0, 1, produce(0, 1)))
                for tch in range(SC):
                    project_v(tch)          # v for sequence 0
                for hp in range(2, DT):
                    nxt = pend.pop(0)
                    pend.append((0, hp, produce(0, hp)))
                    consume(*nxt)
                for tch in range(SC, TC):
                    project_v(tch)          # v for sequence 1
                for hp in range(DT):
                    nxt = pend.pop(0)
                    pend.append((1, hp, produce(1, hp)))
                    consume(*nxt)
                while pend:
                    consume(*pend.pop(0))

            # ---------- Wo + residual + LN1, FFN + residual + LN2 ----
            # Each LayerNorm is split into stats (PE reduction) and finish
            # (scalar chain + apply); finishes are emitted after the other
            # chunk's matmul stage so the chains hide under PE work.
            st1 = []
            for ch in range(NCH):
                for mt in range(DT):
                    ps = ps8.tile([P, CH], FP32, tag="ps")
                    for kt in range(DT):
                        nc.tensor.matmul(
                            ps, lhsT=wo_t[:, mt, kt, :],
                            rhs=o[:, kt, ch * CH:(ch + 1) * CH],
                            start=(kt == 0), stop=(kt == DT - 1))
                    nc.vector.scalar_tensor_tensor(
                        out=h[:, mt, ch * CH:(ch + 1) * CH], in0=ps,
                        scalar=bo_t[:, mt:mt + 1],
                        in1=h[:, mt, ch * CH:(ch + 1) * CH],
                        op0=OP.add, op1=OP.add)
                st1.append(_ln_stats(nc, h, ch, ln_env))
            for ch in range(NCH):
                _ln_finish(nc, h, hb, ch, g1_t, gb1_t, st1[ch], ln_env)

            st2 = []
            for ch in range(NCH):
                # phase 1: ff = relu((h W1 + b1) / FSC), stored fp8 in SBUF
                ff_all = ffp.tile([P, FT, CH], BF16, tag="ffall")
                for m in range(FT):
                    w1t = w1pool.tile([P, DT, P], BF16, tag="w1")
                    nc.sync.dma_start(w1t, io["w1"][l, m])
                    ps = ps8.tile([P, CH], FP32, tag="ps")
                    for kt in range(DT):
                        nc.tensor.matmul(
                            ps, lhsT=w1t[:, kt, :],
                            rhs=hb[:, kt, ch * CH:(ch + 1) * CH],
                            start=(kt == 0), stop=(kt == DT - 1))
                    nc.scalar.activation(ff_all[:, m, :], ps, AF.Relu,
                                         bias=b1_t[:, m:m + 1])
                # phase 2: W2-major accumulation, one PSUM bank per out tile
                for mt in range(DT):
                    w2m = w2pool.tile([P, FT, P], BF16, tag="w2")
                    nc.sync.dma_start(w2m, io["w2"][l, mt])
                    acc = ps8.tile([P, CH], FP32, tag="ps", name="acc")
                    for m in range(FT):
                        nc.tensor.matmul(
                            acc, lhsT=w2m[:, m, :],
                            rhs=ff_all[:, m, :],
                            start=(m == 0), stop=(m == FT - 1))
                    nc.vector.scalar_tensor_tensor(
                        out=h[:, mt, ch * CH:(ch + 1) * CH], in0=acc,
                        scalar=b2_t[:, mt:mt + 1],
                        in1=h[:, mt, ch * CH:(ch + 1) * CH],
                        op0=OP.add, op1=OP.add)
                st2.append(_ln_stats(nc, h, ch, ln_env))
            for ch in range(NCH):
                _ln_finish(nc, h, hb if l < L - 1 else None, ch, g2_t,
                           gb2_t, st2[ch], ln_env)

        nc.sync.dma_start(io["out"][:], h[:].bitcast(FP32))


def _ln_stats(nc, h, ch, ln_env):
    """Phase-1 LayerNorm: per-token mean and mean-square via PE reduction."""
    ones_row128, ones2, inv_d_col, tmp, smalls, ps8 = ln_env
    chs = slice(ch * CH, (ch + 1) * CH)
    mean_ps = ps8.tile([1, CH], FP32, tag="ps", name="mean_ps")
    msq_ps = ps8.tile([1, CH], FP32, tag="ps", name="msq_ps")
    for mt in range(DT):
        sq = tmp.tile([P, CH], FP32R, tag="sq")
        nc.scalar.activation(sq, h[:, mt, chs], AF.Square)
        nc.tensor.matmul(mean_ps[0:1, :], lhsT=inv_d_col,
                         rhs=h[:, mt, chs], start=(mt == 0),
                         stop=(mt == DT - 1))
        nc.tensor.matmul(msq_ps[0:1, :], lhsT=inv_d_col, rhs=sq,
                         start=(mt == 0), stop=(mt == DT - 1))
    return mean_ps, msq_ps


def _ln_finish(nc, h, hb, ch, g_t, gb_t, stats, ln_env):
    """Phase-2 LayerNorm: rstd chain, broadcast, apply (+optional bf16 cast).
    In-place over the feature (partition) axis of h[:, :, chunk]."""
    ones_row128, ones2, inv_d_col, tmp, smalls, ps8 = ln_env
    chs = slice(ch * CH, (ch + 1) * CH)
    mean_ps, msq_ps = stats
    mean_sb = smalls.tile([1, CH], FP32, tag="lnrow", bufs=4, name="mean_sb")
    nc.vector.tensor_copy(mean_sb, mean_ps[0:1, :])
    sqm = smalls.tile([1, CH], FP32, tag="lnrow", bufs=4, name="sqm")
    nc.vector.tensor_tensor(out=sqm, in0=mean_sb, in1=mean_sb, op=OP.mult)
    var = smalls.tile([1, CH], FP32, tag="lnrow", bufs=4, name="var")
    nc.vector.scalar_tensor_tensor(out=var, in0=msq_ps[0:1, :],
                                   scalar=EPS, in1=sqm, op0=OP.add,
                                   op1=OP.subtract)
    lnv = smalls.tile([1, CH], FP32, tag="lnrow", bufs=4, name="lnv")
    nc.scalar.activation(lnv, var, AF.Ln)
    mr = smalls.tile([1, CH], FP32R, tag="lnrow", bufs=4, name="mr")
    nc.scalar.activation(mr, lnv, AF.Exp, scale=-0.5)  # rstd
    # rhs2: row 0 = mean*rstd, row 1 = ones (for the fused K=2 c2 matmul)
    rhs2 = smalls.tile([2, CH], FP32R, tag="rhs2", name="rhs2")
    nc.vector.tensor_copy(rhs2, ones2)
    nc.vector.tensor_tensor(out=rhs2[0:1, :], in0=mean_sb, in1=mr,
                            op=OP.mult)
    rstd_b = ps8.tile([P, CH], FP32, tag="ps", name="rstd_b")
    nc.tensor.matmul(rstd_b, lhsT=ones_row128, rhs=mr, start=True, stop=True)
    for mt in range(DT):
        c2 = ps8.tile([P, CH], FP32, tag="ps", name="c2")
        nc.tensor.matmul(c2, lhsT=gb_t[0:2, mt * P:(mt + 1) * P],
                         rhs=rhs2[0:2, :], start=True, stop=True)
        t2 = tmp.tile([P, CH], FP32, tag="t2")
        nc.vector.tensor_tensor(out=t2, in0=h[:, mt, chs], in1=rstd_b,
                                op=OP.mult)
        nc.vector.scalar_tensor_tensor(
            out=h[:, mt, chs], in0=t2, scalar=g_t[:, mt:mt + 1],
            in1=c2, op0=OP.mult, op1=OP.add)
        if hb is not None:
            nc.gpsimd.tensor_copy(hb[:, mt, chs], h[:, mt, chs])


# ---------------- host side ----------------

def _bf16(a):
    import ml_dtypes
    return np.ascontiguousarray(a).astype(ml_dtypes.bfloat16)


def _fp8(a):
    import ml_dtypes
    return np.ascontiguousarray(a).astype(mybir.dt.np(FP8))


def _pos_encoding_np():
    pos = np.arange(S, dtype=np.float32)[:, None]
    i = np.arange(D // 2, dtype=np.float32)[None, :]
    denom_s = np.power(np.float32(10000.0), (2.0 * i / D).astype(np.float32))
    denom_c = np.power(np.float32(10000.0),
                       (2.0 * (i + 1.0) / D).astype(np.float32))
    pe = np.zeros((S, D), np.float32)
    pe[:, 0::2] = np.sin(pos / denom_s)
    pe[:, 1::2] = np.cos(pos / denom_c)
    return pe  # [S, D]


def _prep_shared(emb, Wq, bq, Wk, bk, Wv, bv, Wo, bo, W1, b1, W2, b2,
                 g1, be1, g2, be2):
    f32 = np.float32
    scale = f32(1.0 / np.sqrt(DN))

    def cols(a, nt):  # [L, nt*128] -> [L, 128, nt]
        return np.ascontiguousarray(
            np.asarray(a).reshape(L, nt, P).transpose(0, 2, 1)).astype(f32)

    def blocks(a):  # [L, D(k), D(m)] -> [L, P(k), DT(mt), DT(kt), P(m)]
        return _bf16(
            a.reshape(L, DT, P, DT, P).transpose(0, 2, 3, 1, 4))

    Wq, Wk, Wv, Wo = (np.asarray(a)[:L] for a in (Wq, Wk, Wv, Wo))
    W1, W2 = np.asarray(W1)[:L], np.asarray(W2)[:L]
    bq, bk, bv, bo = (np.asarray(a)[:L] for a in (bq, bk, bv, bo))
    b1, b2 = np.asarray(b1)[:L], np.asarray(b2)[:L]
    g1, be1, g2, be2 = (np.asarray(a)[:L] for a in (g1, be1, g2, be2))

    wq_h = blocks(Wq.transpose(0, 2, 1, 3).reshape(L, D, D) * scale)
    wk_h = blocks(Wk.transpose(0, 2, 1, 3).reshape(L, D, D))
    wo_h = blocks(Wo.astype(f32))
    wv_h = _bf16(Wv.transpose(0, 2, 1, 3).reshape(L, DT, P, D)
                 .transpose(0, 2, 1, 3))
    w1_h = _bf16(W1.reshape(L, DT, P, FT, P).transpose(0, 3, 2, 1, 4))
    w2_h = _bf16(W2.reshape(L, FT, P, DT, P)
                 .transpose(0, 3, 2, 1, 4))

    return dict(
        wq=wq_h, wk=wk_h, wv=wv_h, wo=wo_h, w1=w1_h, w2=w2_h,
        bq_c=cols(bq.reshape(L, D) * scale, DT),
        bk_c=cols(bk.reshape(L, D), DT),
        bv_r=np.ascontiguousarray(bv.reshape(L, D)).astype(f32),
        bo_c=cols(bo, DT),
        b1_c=cols(b1, FT),
        b2_c=cols(b2, DT),
        g1_c=cols(g1, DT),
        g2_c=cols(g2, DT),
        gb1=np.ascontiguousarray(np.stack([-g1, be1], axis=1)).astype(f32),
        gb2=np.ascontiguousarray(np.stack([-g2, be2], axis=1)).astype(f32),
    )


def _prep_percore(x, padding_mask, emb):
    """Per-core h0 (= emb[x] + pos-encoding, feature-major) and mask."""
    emb = np.asarray(emb, dtype=np.float32)
    x_i = np.asarray(x).astype(np.int64)
    mask_f = (1.0 - np.asarray(padding_mask).astype(np.float32))
    pe = _pos_encoding_np()                       # [S, D]
    per = []
    for c in range(NCORES):
        xs = x_i[c * BL:(c + 1) * BL]             # [BL, S]
        hc = emb[xs.reshape(-1)] + np.tile(pe, (BL, 1))   # [T, D]
        h0 = np.ascontiguousarray(
            hc.T.reshape(DT, P, T).transpose(1, 0, 2)).astype(np.float32)
        ms = mask_f[c * BL:(c + 1) * BL]          # [BL, S, S]
        # mmask[p, b, tci, s] = (1-mask)[b, s, tci*128+p]
        mt = _fp8(
            ms.transpose(0, 2, 1).reshape(BL, SC, P, S).transpose(2, 0, 1, 3))
        per.append(dict(h0=h0, mmask=mt))
    return per


def kernel(x, padding_mask, emb, Wq, bq, Wk, bk, Wv, bv, Wo, bo,
           W1, b1, W2, b2, g1, be1, g2, be2):
    if "nc" not in _PROGRAM_CACHE:
        _PROGRAM_CACHE["nc"] = _build_program(reps=1)
    nc = _PROGRAM_CACHE["nc"]

    shared = _prep_shared(emb, Wq, bq, Wk, bk, Wv, bv, Wo, bo, W1, b1, W2, b2,
                          g1, be1, g2, be2)
    per = _prep_percore(x, padding_mask, emb)

    in_maps = []
    for c in range(NCORES):
        m = dict(shared)
        m.update(per[c])
        in_maps.append(m)

    res = run_bass_kernel_spmd(nc, in_maps, core_ids=list(range(NCORES)))

    outs = []
    for c in range(NCORES):
        oc = res.results[c]["out"]                    # [P, DT, T]
        hc = oc.transpose(2, 1, 0).reshape(T, D)      # [T, D]
        outs.append(hc.reshape(BL, S, D))
    return np.concatenate(outs, axis=0).astype(np.float32)


if __name__ == "__main__":
    pass
